# revision 27
# baseline (speedup 1.0000x reference)
"""Trainium2 Bass kernel for nn_EvolvingGNN (LSTM-evolved GCN + edge MLP).

Strategy (8 NeuronCores, full inputs in / full output out):
  - Nodes sharded 12500/core. Edges partitioned by destination core.
  - LSTM distributed: each core computes 512 of the 4096 gate rows
    (reads only its slice of W_ih/W_hh), AllGather of h each step.
  - xwd[n] = dinv[n] * (x[n] @ W) computed on the node shard, AllGathered
    into a full 256B-row table for gathers.
  - Message phase: dma_gather xwd[src] -> dma_scatter_add into agg[dst]
    (CCE add). Scatter calls must have unique indices (duplicate rows in
    one call race on read-modify-write), so edges are organised into
    "rounds" (r-th in-edge of each node) with round-robin over 4
    accumulator tables to hide the inter-round ordering latency.
  - emb = relu(dinv * (agg + xwd_self)); uv = [emb@W1a.T | emb@W1b.T]
    (one 256B row per node), AllGathered.
  - Edge MLP: gather uv[src] (u half) + uv[dst] (v half), w = ea@W1c.T+b1
    via PE matmuls on host-transposed edge_attr, logits = relu(z) . W2 + b2
    via DVE mul+reduce.
  - Gather indices are int16, so the node-table rows are bucketed in
    32768-row groups; the per-core edge order is (bucket, round, dst).
    Pads: gathers use row 0, scatters use a trash row.

Wall-clock optimisations (the axon tunnel moves ~60MB/s, so host->device
bytes dominate the end-to-end time):
  - Accumulator tables and uv_own are Internal DRAM zeroed on device
    (previously ExternalOutputs: ~13MB/core of donated zeros uploaded and
    ~16MB/core of unused outputs downloaded per call).
  - Big payloads (edge features, x, LSTM weights) travel as bfloat16;
    matmuls run bf16 x bf16 -> f32 PSUM.
  - Gather/scatter index planes are sent as the 16-partition master copy
    and replicated to the 128-partition layout on device (8x fewer bytes).
  - host_prep is vectorised: one combined-key argsort pipeline over all
    edges instead of per-core lexsorts.
"""

import os
import pickle
import subprocess
import sys

import numpy as np
import ml_dtypes

import concourse.bacc as bacc
import concourse.mybir as mybir
import concourse.tile as tile
from concourse.bass_utils import run_bass_kernel_spmd
from concourse.masks import make_identity

F32 = mybir.dt.float32
BF16 = mybir.dt.bfloat16
I16 = mybir.dt.int16
NPBF16 = ml_dtypes.bfloat16


class CFG:
    def __init__(self, N, E, T, DIN, DH, EF, NC=8, CHUNK=8192, CCH=4):
        self.N, self.E, self.T = N, E, T
        self.DIN, self.DH, self.EF = DIN, DH, EF
        self.FLAT = DIN * DH
        self.NC = NC
        assert N % NC == 0
        self.SH = N // NC                       # nodes per core
        self.TILES = -(-self.SH // 128)         # node tiles per core
        self.SHP = self.TILES * 128             # padded shard rows
        self.NTAB = NC * self.SHP               # full table rows
        self.NBUCK = -(-self.NTAB // 32768)
        self.CHUNK = CHUNK                      # gather chunk (edges)
        self.CCH = CCH                          # scatter chain tables
        self.ROW = 64                           # table row f32 (256B)
        # LSTM slicing: core k owns gate rows {g*FLAT + k*GSL + j}
        assert (4 * self.FLAT) % NC == 0
        self.GSL = self.FLAT // NC              # per-gate slice (128)
        self.KCH = self.FLAT // 128             # contraction chunks (8)


def _roundup(x, m):
    return -(-x // m) * m


# ---------------------------------------------------------------------------
# Parallel host prep: 8 numpy-only worker subprocesses over shared memory.
# Phase A: per-core edge sort into (bucket, round, dst) order + round counts.
# Phase B: slot assignment + assembly of idx planes / edge features / weights.
# ---------------------------------------------------------------------------

_WORKER_SRC = r"""
import sys, pickle
import numpy as np
import ml_dtypes
from multiprocessing import shared_memory

BF16 = ml_dtypes.bfloat16
_inp = sys.stdin.buffer
_out = sys.stdout.buffer
_shm = {}
_state = {}


def att(name):
    s = _shm.get(name)
    if s is None:
        s = shared_memory.SharedMemory(name=name, track=False)
        _shm[name] = s
    return s


def view(name, shape, dtype):
    n = int(np.prod(shape)) * np.dtype(dtype).itemsize
    return np.ndarray(shape, dtype, buffer=att(name).buf[:n])


while True:
    try:
        cmd = pickle.load(_inp)
    except EOFError:
        break
    op = cmd["op"]
    if op == "A":
        k = cmd["k"]; E = cmd["E"]; SH = cmd["SH"]; SHP = cmd["SHP"]
        NBUCK = cmd["NBUCK"]
        ei = view(cmd["ei"], (2, E), np.int32)
        dst = ei[1]
        lo = k * SH
        eids = np.flatnonzero((dst >= lo) & (dst < lo + SH))
        n = len(eids)
        s = ei[0][eids].astype(np.int64)
        rowid = (s // SH) * SHP + (s % SH)
        sbuck = (rowid >> 15).astype(np.int32)
        s16 = (rowid & 32767).astype(np.int16)
        dloc = (dst[eids] - lo).astype(np.int32)
        o1 = np.argsort(sbuck * np.int32(SH) + dloc, kind="stable")
        b1, d1 = sbuck[o1], dloc[o1]
        k1 = b1 * np.int32(SH) + d1
        newrun = np.empty(n, bool); newrun[:1] = True
        np.not_equal(k1[1:], k1[:-1], out=newrun[1:])
        starts = np.flatnonzero(newrun)
        r1 = (np.arange(n) - np.repeat(starts, np.diff(np.r_[starts, n]))).astype(np.int32)
        MAXR = int(r1.max()) + 1 if n else 1
        o2 = np.argsort((b1 * np.int32(MAXR) + r1) * np.int32(SH) + d1,
                        kind="stable")
        _state["eids"] = eids[o1][o2]
        _state["b"] = b1[o2]
        _state["r"] = r1[o2]
        _state["d"] = d1[o2]
        _state["s16"] = s16[o1][o2]
        _state["MAXR"] = MAXR
        cnt = np.bincount(_state["b"] * np.int32(MAXR) + _state["r"],
                          minlength=NBUCK * MAXR).reshape(NBUCK, MAXR)
        pickle.dump(cnt, _out); _out.flush()
    elif op == "B":
        k = cmd["k"]; E = cmd["E"]; TOT = cmd["TOT"]; EF = cmd["EF"]
        NEF = cmd["NEF"]
        SHP = cmd["SHP"]; FLAT = cmd["FLAT"]; GSL = cmd["GSL"]; KCH = cmd["KCH"]
        seg_off = cmd["seg_off"]                       # [NBUCK, MAXR_glob]
        b, r, d = _state["b"], _state["r"], _state["d"]
        eids, s16, MAXR = _state["eids"], _state["s16"], _state["MAXR"]
        n = len(eids)
        ckey = b * np.int32(MAXR) + r
        newseg = np.empty(n, bool); newseg[:1] = True
        np.not_equal(ckey[1:], ckey[:-1], out=newseg[1:])
        sstarts = np.flatnonzero(newseg)
        rank = np.arange(n) - np.repeat(sstarts, np.diff(np.r_[sstarts, n]))
        slot = seg_off[b, r] + rank
        planes = view(cmd["planes"], (8, 2, 16, TOT // 16), np.int16)
        eaT = view(cmd["eaT"], (8, NEF, TOT), BF16)
        orig = view(cmd["orig"], (8, TOT), np.int32)
        wout = view(cmd["wout"], (8, 2, 128, KCH * 4 * GSL), BF16)
        ea = view(cmd["ea"], (E, EF), np.float32)
        u16 = np.zeros(TOT, np.int16); u16[slot] = s16
        vs = np.full(TOT, SHP, np.int16); vs[slot] = d.astype(np.int16)
        planes[k, 0] = u16.reshape(TOT // 16, 16).T
        planes[k, 1] = vs.reshape(TOT // 16, 16).T
        og = np.full(TOT, -1, np.int32); og[slot] = eids.astype(np.int32)
        orig[k] = og
        rows = np.zeros((TOT, NEF), BF16)
        rows[slot, : EF] = ea[eids].astype(BF16)
        if NEF > EF:
            rows[slot, EF] = 1.0
        eaT[k] = rows.T
        grows = np.concatenate([g * FLAT + k * GSL + np.arange(GSL)
                                for g in range(4)])
        for i, nm in enumerate(("wih", "whh")):
            W = view(cmd[nm], (4 * FLAT, FLAT), np.float32)
            wt = W[grows].T
            wout[k, i] = wt.reshape(KCH, 128, 4 * GSL).transpose(1, 0, 2).reshape(
                128, -1).astype(BF16)
        pickle.dump(k, _out); _out.flush()
"""


class _PrepPool:
    def __init__(self, n=8):
        self.n = n
        self.procs = [
            subprocess.Popen([sys.executable, "-u", "-c", _WORKER_SRC],
                             stdin=subprocess.PIPE, stdout=subprocess.PIPE)
            for _ in range(n)
        ]
        self.shms = {}

    def arr(self, key, shape, dtype):
        from multiprocessing import shared_memory
        nbytes = int(np.prod(shape)) * np.dtype(dtype).itemsize
        cur = self.shms.get(key)
        if cur is None or cur.size < nbytes:
            if cur is not None:
                try:
                    cur.close(); cur.unlink()
                except Exception:
                    pass
            cur = shared_memory.SharedMemory(create=True, size=nbytes)
            self.shms[key] = cur
        return np.ndarray(shape, dtype, buffer=cur.buf[:nbytes]), cur.name

    def send(self, i, obj):
        pickle.dump(obj, self.procs[i].stdin)
        self.procs[i].stdin.flush()

    def recv(self, i):
        return pickle.load(self.procs[i].stdout)

    def kill(self):
        for p in self.procs:
            try:
                p.kill()
            except Exception:
                pass
        for s in self.shms.values():
            try:
                s.close(); s.unlink()
            except Exception:
                pass


_POOL = None


def _get_pool():
    global _POOL
    if _POOL is None:
        _POOL = _PrepPool()
    return _POOL


def _host_prep_parallel(inputs, cfg):
    c = cfg
    pool = _get_pool()
    ei = np.asarray(inputs["edge_index"])
    ei_shm, ei_name = pool.arr("ei", (2, c.E), np.int32)
    np.copyto(ei_shm, ei)
    ea_shm, ea_name = pool.arr("ea", (c.E, c.EF), np.float32)
    np.copyto(ea_shm, np.asarray(inputs["edge_attr"], np.float32))
    wih_shm, wih_name = pool.arr("wihf", (4 * c.FLAT, c.FLAT), np.float32)
    np.copyto(wih_shm, np.asarray(inputs["W_ih"], np.float32))
    whh_shm, whh_name = pool.arr("whhf", (4 * c.FLAT, c.FLAT), np.float32)
    np.copyto(whh_shm, np.asarray(inputs["W_hh"], np.float32))
    for k in range(c.NC):
        pool.send(k, {"op": "A", "k": k, "E": c.E, "SH": c.SH, "SHP": c.SHP,
                      "NBUCK": c.NBUCK, "ei": ei_name})

    # parent-side smalls while workers sort
    x_last = np.asarray(inputs["x"][-1], np.float32)
    xlT16 = x_last.T.astype(NPBF16)                             # [DIN, N]
    dst = ei_shm[1]
    deg = np.bincount(dst, minlength=c.N).astype(np.float32) + 1.0
    dinv = (1.0 / np.sqrt(deg)).astype(np.float32)
    W1 = np.asarray(inputs["W1"], np.float32)
    w1ab = np.ascontiguousarray(
        np.concatenate([W1[:, : c.DH].T, W1[:, c.DH : 2 * c.DH].T], axis=1))
    b1v = np.asarray(inputs["b1"], np.float32)
    has_b1 = bool(np.any(b1v))
    NEF = c.EF + 1 if has_b1 else c.EF
    w1c_parts = [W1[:, 2 * c.DH :].T] + ([b1v[None, :]] if has_b1 else [])
    w1c = np.ascontiguousarray(np.concatenate(w1c_parts).astype(NPBF16))
    w2 = np.asarray(inputs["W2"], np.float32).reshape(-1)
    w2row = np.ascontiguousarray(np.tile(w2, 512 // c.DH)[None, :])
    w0 = np.asarray(inputs["initial_weights"], np.float32).reshape(-1)
    w0t = np.ascontiguousarray(w0.reshape(c.KCH, 128).T.astype(NPBF16))
    b_ih = np.asarray(inputs["b_ih"], np.float32)
    b_hh = np.asarray(inputs["b_hh"], np.float32)

    cnts = [pool.recv(k) for k in range(c.NC)]
    MAXR = max(cn.shape[1] for cn in cnts)
    segmax = np.zeros((c.NBUCK, MAXR), np.int64)
    for cn in cnts:
        np.maximum(segmax[:, : cn.shape[1]], cn, out=segmax[:, : cn.shape[1]])
    segsz = np.where(segmax > 0, ((segmax + 127) // 128) * 128, 0).astype(np.int64)
    seg_off = np.concatenate([[0], np.cumsum(segsz.reshape(-1))])[:-1].reshape(
        c.NBUCK, MAXR)
    TOT = int(segsz.sum())

    blen = segsz.sum(axis=1)
    bstarts = np.concatenate([[0], np.cumsum(blen)])
    pieces = []
    piece_ctr = 0
    for bb in range(c.NBUCK):
        bstart, bl = int(bstarts[bb]), int(blen[bb])
        if bl == 0:
            continue
        cuts = list(range(bstart, bstart + bl, c.CHUNK)) + [bstart + bl]
        for ci in range(len(cuts) - 1):
            coff, cend = cuts[ci], cuts[ci + 1]
            plist = []
            for rv in range(MAXR):
                if segsz[bb, rv] == 0:
                    continue
                so = int(seg_off[bb, rv])
                se = so + int(segsz[bb, rv])
                lo, hi = max(so, coff), min(se, cend)
                while lo < hi:
                    sub = min(hi - lo, 4096)
                    plist.append((lo - coff, sub, piece_ctr % c.CCH))
                    piece_ctr += 1
                    lo += sub
            pieces.append((bb, coff, cend - coff, plist))

    planes_shm, planes_name = pool.arr("planes", (8, 2, 16, TOT // 16), np.int16)
    eaT_shm, eaT_name = pool.arr("eaTo", (8, NEF, TOT), NPBF16)
    orig_shm, orig_name = pool.arr("orig", (8, TOT), np.int32)
    wout_shm, wout_name = pool.arr("wout", (8, 2, 128, c.KCH * 4 * c.GSL), NPBF16)
    for k in range(c.NC):
        pool.send(k, {"op": "B", "k": k, "E": c.E, "TOT": TOT, "EF": c.EF,
                      "NEF": NEF,
                      "SHP": c.SHP, "FLAT": c.FLAT, "GSL": c.GSL, "KCH": c.KCH,
                      "seg_off": seg_off, "planes": planes_name,
                      "eaT": eaT_name, "orig": orig_name, "wout": wout_name,
                      "ea": ea_name, "wih": wih_name, "whh": whh_name})

    in_maps = []
    for k in range(c.NC):
        n0 = k * c.SH
        xT = np.zeros((c.DIN, c.SHP), NPBF16)
        xT[:, : c.SH] = xlT16[:, n0 : n0 + c.SH]
        dflat = np.ones(c.SHP, np.float32)
        dflat[: c.SH] = dinv[n0 : n0 + c.SH]
        dvt = np.ascontiguousarray(dflat.reshape(c.TILES, 128).T)
        rows = np.concatenate(
            [g * c.FLAT + k * c.GSL + np.arange(c.GSL) for g in range(4)])
        bsl = np.concatenate(
            [b_ih[rows].reshape(4, c.GSL).T, b_hh[rows].reshape(4, c.GSL).T],
            axis=1)
        in_maps.append({
            "xT": xT, "dinv": dvt,
            "wih": wout_shm[k, 0], "whh": wout_shm[k, 1],
            "bsl": np.ascontiguousarray(bsl), "w0": w0t,
            "w1ab": w1ab, "w1c": w1c, "w2row": w2row,
            "uidx": planes_shm[k, 0], "vsidx": planes_shm[k, 1],
            "eaT": eaT_shm[k],
        })
    for k in range(c.NC):
        pool.recv(k)

    struct = {
        "TOT": TOT,
        "NEF": NEF,
        "pieces": pieces,
        "b2": float(np.asarray(inputs["b2"], np.float32).reshape(-1)[0]),
    }
    return in_maps, struct, orig_shm.reshape(-1)


def _ncpu():
    try:
        return len(os.sched_getaffinity(0))
    except Exception:
        return os.cpu_count() or 1


def host_prep(inputs, cfg):
    # The worker pool only pays off with real parallelism; on the 1-2 CPU
    # containers the serial vectorised path is strictly better.
    global _POOL
    if os.environ.get("KPREP_SERIAL") != "1" and _ncpu() >= 4:
        try:
            return _host_prep_parallel(inputs, cfg)
        except Exception:
            if _POOL is not None:
                _POOL.kill()
                _POOL = None
    return _host_prep_serial(inputs, cfg)


def _host_prep_serial(inputs, cfg):
    """Shard / reorder everything on the host. Returns (in_maps, struct, origs)."""
    c = cfg
    x_last = np.asarray(inputs["x"][-1], np.float32)            # [N, DIN]
    ei = np.asarray(inputs["edge_index"])                       # [2, E]
    ea = np.asarray(inputs["edge_attr"], np.float32)            # [E, EF]
    src = ei[0].astype(np.int32)
    dst = ei[1].astype(np.int32)

    deg = np.bincount(dst, minlength=c.N).astype(np.float32) + 1.0
    dinv = (1.0 / np.sqrt(deg)).astype(np.float32)

    rowid = (src // c.SH) * c.SHP + (src % c.SH)                # table row of src
    sbuck = rowid >> 15
    s16 = (rowid & 32767).astype(np.int16)
    ecore = dst // c.SH
    dloc = dst - ecore * c.SH

    # ---- global (core, bucket, round, dst) ordering ----
    key1 = (ecore * c.NBUCK + sbuck) * c.SH + dloc              # int32
    o1 = np.argsort(key1, kind="stable").astype(np.int32)
    k1 = key1[o1]
    newrun = np.empty(c.E, bool)
    newrun[0] = True
    np.not_equal(k1[1:], k1[:-1], out=newrun[1:])
    starts = np.flatnonzero(newrun).astype(np.int32)
    ar = np.arange(c.E, dtype=np.int32)
    r1 = ar - np.repeat(starts, np.diff(np.r_[starts, np.int32(c.E)]))
    MAXR = int(r1.max()) + 1
    key2 = ((ecore[o1] * c.NBUCK + sbuck[o1]) * np.int32(MAXR) + r1) * c.SH \
        + dloc[o1]
    o2 = np.argsort(key2, kind="stable").astype(np.int32)
    eid2 = o1[o2]
    ec2, b2v, r2, d2 = ecore[eid2], sbuck[eid2], r1[o2], dloc[eid2]

    # ---- universal segment sizes: max count over cores per (bucket, round) ----
    ckey = (ec2 * c.NBUCK + b2v) * np.int32(MAXR) + r2
    cnt = np.bincount(ckey, minlength=c.NC * c.NBUCK * MAXR).reshape(
        c.NC, c.NBUCK, MAXR)
    segmax = cnt.max(axis=0)                                    # [NBUCK, MAXR]
    segsz = np.where(segmax > 0, ((segmax + 127) // 128) * 128, 0).astype(np.int64)
    seg_off = np.concatenate([[0], np.cumsum(segsz.reshape(-1))])[:-1].reshape(
        c.NBUCK, MAXR).astype(np.int32)
    TOT = int(segsz.sum())
    assert TOT % 128 == 0

    # ---- per-edge slot ----
    newseg = np.empty(c.E, bool)
    newseg[0] = True
    np.not_equal(ckey[1:], ckey[:-1], out=newseg[1:])
    sstarts = np.flatnonzero(newseg).astype(np.int32)
    rank = ar - np.repeat(sstarts, np.diff(np.r_[sstarts, np.int32(c.E)]))
    slot = seg_off[b2v, r2] + rank                              # [0, TOT) per core
    gslot = ec2 * np.int32(TOT) + slot

    # ---- chunk / scatter-piece structure (identical for all cores) ----
    blen = segsz.sum(axis=1)                                    # per bucket
    bstarts = np.concatenate([[0], np.cumsum(blen)])
    pieces = []                                                 # (bb,coff,clen,[(po,pl,chain)])
    piece_ctr = 0
    for bb in range(c.NBUCK):
        bstart, bl = int(bstarts[bb]), int(blen[bb])
        if bl == 0:
            continue
        cuts = list(range(bstart, bstart + bl, c.CHUNK)) + [bstart + bl]
        for ci in range(len(cuts) - 1):
            coff, cend = cuts[ci], cuts[ci + 1]
            plist = []
            for rv in range(MAXR):
                if segsz[bb, rv] == 0:
                    continue
                so = int(seg_off[bb, rv])
                se = so + int(segsz[bb, rv])
                lo, hi = max(so, coff), min(se, cend)
                # dma_scatter_add breaks above 4096 idxs per call
                while lo < hi:
                    sub = min(hi - lo, 4096)
                    plist.append((lo - coff, sub, piece_ctr % c.CCH))
                    piece_ctr += 1
                    lo += sub
            pieces.append((bb, coff, cend - coff, plist))

    # ---- global slot-order tables ----
    TRASH = c.SHP                                               # scatter/v pad row
    NT = c.NC * TOT
    u16_all = np.zeros(NT, np.int16)
    u16_all[gslot] = s16[eid2]
    vs_all = np.full(NT, TRASH, np.int16)
    vs_all[gslot] = d2.astype(np.int16)
    orig_all = np.full(NT, -1, np.int32)
    orig_all[gslot] = eid2

    b1v = np.asarray(inputs["b1"], np.float32)
    has_b1 = bool(np.any(b1v))
    NEF = c.EF + 1 if has_b1 else c.EF
    ea16 = ea.astype(NPBF16)
    ea_rows = np.zeros((NT, NEF), NPBF16)
    ea_rows[gslot, : c.EF] = ea16[eid2]
    if has_b1:
        ea_rows[gslot, c.EF] = 1.0

    xlT16 = x_last.T.astype(NPBF16)                             # [DIN, N]

    W1 = np.asarray(inputs["W1"], np.float32)                   # [DH, 2DH+EF]
    w1ab = np.ascontiguousarray(
        np.concatenate([W1[:, : c.DH].T, W1[:, c.DH : 2 * c.DH].T], axis=1))
    w1c_parts = [W1[:, 2 * c.DH :].T] + ([b1v[None, :]] if has_b1 else [])
    w1c = np.ascontiguousarray(np.concatenate(w1c_parts).astype(NPBF16))
    w2 = np.asarray(inputs["W2"], np.float32).reshape(-1)       # [DH]
    w2row = np.ascontiguousarray(np.tile(w2, 512 // c.DH)[None, :])  # [1, 512]
    w0 = np.asarray(inputs["initial_weights"], np.float32).reshape(-1)
    w0t = np.ascontiguousarray(w0.reshape(c.KCH, 128).T.astype(NPBF16))
    W_ih = np.asarray(inputs["W_ih"], np.float32)
    W_hh = np.asarray(inputs["W_hh"], np.float32)
    b_ih = np.asarray(inputs["b_ih"], np.float32)
    b_hh = np.asarray(inputs["b_hh"], np.float32)

    in_maps = []
    for k in range(c.NC):
        sl = slice(k * TOT, (k + 1) * TOT)
        n0 = k * c.SH

        xT = np.zeros((c.DIN, c.SHP), NPBF16)
        xT[:, : c.SH] = xlT16[:, n0 : n0 + c.SH]
        dflat = np.ones(c.SHP, np.float32)
        dflat[: c.SH] = dinv[n0 : n0 + c.SH]
        dvt = np.ascontiguousarray(dflat.reshape(c.TILES, 128).T)

        rows = np.concatenate(
            [g * c.FLAT + k * c.GSL + np.arange(c.GSL) for g in range(4)])

        def wl(w):
            wt = w[rows].T                                      # [FLAT, 4*GSL]
            return np.ascontiguousarray(
                wt.reshape(c.KCH, 128, 4 * c.GSL).transpose(1, 0, 2).reshape(
                    128, c.KCH * 4 * c.GSL).astype(NPBF16))

        bsl = np.concatenate(
            [b_ih[rows].reshape(4, c.GSL).T, b_hh[rows].reshape(4, c.GSL).T],
            axis=1)                                             # [GSL, 8]

        in_maps.append({
            "xT": xT,
            "dinv": dvt,
            "wih": wl(W_ih),
            "whh": wl(W_hh),
            "bsl": np.ascontiguousarray(bsl),
            "w0": w0t,
            "w1ab": w1ab,
            "w1c": w1c,
            "w2row": w2row,
            "uidx": np.ascontiguousarray(u16_all[sl].reshape(TOT // 16, 16).T),
            "vsidx": np.ascontiguousarray(vs_all[sl].reshape(TOT // 16, 16).T),
            "eaT": np.ascontiguousarray(ea_rows[sl].T),         # [NEF, TOT] bf16
        })

    struct = {
        "TOT": TOT,
        "NEF": NEF,
        "pieces": pieces,
        "b2": float(np.asarray(inputs["b2"], np.float32).reshape(-1)[0]),
    }
    return in_maps, struct, orig_all


def build(cfg, struct):
    c = cfg
    TOT = struct["TOT"]
    NEF = struct["NEF"]
    nc = bacc.Bacc("TRN2", target_bir_lowering=False, debug=False,
                   num_devices=c.NC)

    # ---------- I/O ----------
    xT_h = nc.dram_tensor("xT", [c.DIN, c.SHP], BF16, kind="ExternalInput")
    dinv_h = nc.dram_tensor("dinv", [128, c.TILES], F32, kind="ExternalInput")
    wih_h = nc.dram_tensor("wih", [128, c.KCH * 4 * c.GSL], BF16, kind="ExternalInput")
    whh_h = nc.dram_tensor("whh", [128, c.KCH * 4 * c.GSL], BF16, kind="ExternalInput")
    bsl_h = nc.dram_tensor("bsl", [c.GSL, 8], F32, kind="ExternalInput")
    w0_h = nc.dram_tensor("w0", [128, c.KCH], BF16, kind="ExternalInput")
    w1ab_h = nc.dram_tensor("w1ab", [c.DH, 2 * c.DH], F32, kind="ExternalInput")
    w1c_h = nc.dram_tensor("w1c", [NEF, c.DH], BF16, kind="ExternalInput")
    w2row_h = nc.dram_tensor("w2row", [1, 512], F32, kind="ExternalInput")
    uidx_h = nc.dram_tensor("uidx", [16, TOT // 16], I16, kind="ExternalInput")
    vsidx_h = nc.dram_tensor("vsidx", [16, TOT // 16], I16, kind="ExternalInput")
    eaT_h = nc.dram_tensor("eaT", [NEF, TOT], BF16, kind="ExternalInput")

    logits_h = nc.dram_tensor("logits", [128, TOT // 128], BF16, kind="ExternalOutput")
    # internal accumulator tables, zeroed on device before the scatter phase
    aggs = [nc.dram_tensor(f"agg{i}", [c.SHP + 128, c.ROW], F32)
            for i in range(c.CCH)]
    uv_own = nc.dram_tensor("uv_own", [c.SHP + 128, c.ROW], F32)

    # internal DRAM
    xwd_own = nc.dram_tensor("xwd_own", [c.SHP, c.ROW], F32)
    xwd_full = nc.dram_tensor("xwd_full", [c.NTAB, c.ROW], F32, addr_space="Shared")
    uv_shard = nc.dram_tensor("uv_shard", [c.SHP, c.ROW], F32)
    uv_full = nc.dram_tensor("uv_full", [c.NTAB, c.ROW], F32, addr_space="Shared")
    hb_in = nc.dram_tensor("hb_in", [128, 1], F32)
    hb_out = nc.dram_tensor("hb_out", [c.FLAT, 1], F32)

    groups = [list(range(c.NC))]

    with tile.TileContext(nc) as tc:
        with (
            tc.tile_pool(name="persist", bufs=1) as pp,
            tc.tile_pool(name="psum_ls", bufs=2, space="PSUM") as ps_ls,
        ):
            # ---------- persistent small tiles ----------
            ident = pp.tile([128, 128], F32)
            make_identity(nc, ident[:])
            w1ab_sb = pp.tile([c.DH, 2 * c.DH], F32)
            nc.sync.dma_start(w1ab_sb[:], w1ab_h[:])
            w1c_sb = pp.tile([NEF, c.DH], BF16)
            nc.sync.dma_start(w1c_sb[:], w1c_h[:])
            dinv_sb = pp.tile([128, c.TILES], F32)
            nc.sync.dma_start(dinv_sb[:], dinv_h[:])
            xwd_sb = pp.tile([128, c.TILES, c.DH], F32)  # persists to post-agg
            W_sb = pp.tile([c.DIN, c.DH], BF16)          # evolved GCN weight

            # w2 broadcast [1,512] -> [128,512] via K=1 matmul with ones
            w2r_sb = pp.tile([1, 512], F32)
            nc.sync.dma_start(w2r_sb[:], w2row_h[:])
            ones1 = pp.tile([1, 128], F32)
            nc.vector.memset(ones1[:], 1.0)
            w2_sb = pp.tile([128, 512], F32)
            pw2 = ps_ls.tile([128, 512], F32, tag="w2bc")
            nc.tensor.matmul(pw2[:], ones1[:], w2r_sb[:], start=True, stop=True)
            nc.vector.tensor_copy(w2_sb[:], pw2[:])

            # ---------- zero the accumulator tables (device-side) ----------
            zt = pp.tile([128, 16, c.ROW], F32)
            nc.vector.memset(zt[:], 0.0)
            ntile = (c.SHP + 128) // 128
            for t in aggs:
                av = t[:, :].rearrange("(x p) c -> p x c", p=128)
                for x0 in range(0, ntile, 16):
                    xl = min(16, ntile - x0)
                    nc.sync.dma_start(av[:, x0 : x0 + xl, :], zt[:, :xl, :])
            nc.sync.dma_start(uv_own[c.SHP : c.SHP + 128, :], zt[:, 0, :])

            # ---------- phase 0: distributed LSTM ----------
            with tc.tile_pool(name="lstm", bufs=1) as lp:
                wih_sb = lp.tile([128, c.KCH * 4 * c.GSL], BF16)
                whh_sb = lp.tile([128, c.KCH * 4 * c.GSL], BF16)
                nc.sync.dma_start(wih_sb[:], wih_h[:])
                nc.sync.dma_start(whh_sb[:], whh_h[:])
                bsl_sb = lp.tile([c.GSL, 8], F32)
                nc.sync.dma_start(bsl_sb[:], bsl_h[:])
                bsum = lp.tile([c.GSL, 4], F32)
                nc.vector.tensor_tensor(bsum[:], bsl_sb[:, 0:4], bsl_sb[:, 4:8],
                                        op=mybir.AluOpType.add)
                inp = lp.tile([128, c.KCH], BF16)
                nc.sync.dma_start(inp[:], w0_h[:])
                inpf = lp.tile([128, c.KCH], F32)
                cstate = lp.tile([c.GSL, 1], F32)
                gsb = lp.tile([c.GSL, 4], F32)
                ifgo = lp.tile([c.GSL, 4], F32)
                tmp = lp.tile([c.GSL, 2], F32)
                Wf = lp.tile([c.DIN, c.DH], F32)

                wv = wih_sb[:].rearrange("p (c n) -> p c n", c=c.KCH)
                wsumv = whh_sb[:].rearrange("p (c n) -> p c n", c=c.KCH)

                for step in range(c.T):
                    wmat = wv if step == 0 else wsumv
                    gp = ps_ls.tile([c.GSL, 4], F32, tag="gates")
                    for g in range(4):
                        for kc in range(c.KCH):
                            nc.tensor.matmul(
                                gp[:, g : g + 1],
                                wmat[:, kc, g * c.GSL : (g + 1) * c.GSL],
                                inp[:, kc : kc + 1],
                                start=(kc == 0),
                                stop=(kc == c.KCH - 1),
                            )
                    if step == 0:
                        # wsum = wih + whh (for steps 2..T), overwrite whh
                        nc.vector.tensor_tensor(whh_sb[:], wih_sb[:], whh_sb[:],
                                                op=mybir.AluOpType.add)
                    nc.vector.tensor_tensor(gsb[:], gp[:], bsum[:],
                                            op=mybir.AluOpType.add)
                    Sig = mybir.ActivationFunctionType.Sigmoid
                    Tanh = mybir.ActivationFunctionType.Tanh
                    nc.scalar.activation(ifgo[:, 0:1], gsb[:, 0:1], Sig)
                    nc.scalar.activation(ifgo[:, 1:2], gsb[:, 1:2], Sig)
                    nc.scalar.activation(ifgo[:, 2:3], gsb[:, 2:3], Tanh)
                    nc.scalar.activation(ifgo[:, 3:4], gsb[:, 3:4], Sig)
                    # c' = f*c + i*g ; h' = o * tanh(c')
                    nc.vector.tensor_tensor(tmp[:, 0:1], ifgo[:, 0:1], ifgo[:, 2:3],
                                            op=mybir.AluOpType.mult)
                    if step == 0:
                        nc.vector.tensor_copy(cstate[:], tmp[:, 0:1])
                    else:
                        nc.vector.tensor_tensor(tmp[:, 1:2], ifgo[:, 1:2], cstate[:],
                                                op=mybir.AluOpType.mult)
                        nc.vector.tensor_tensor(cstate[:], tmp[:, 0:1], tmp[:, 1:2],
                                                op=mybir.AluOpType.add)
                    nc.scalar.activation(tmp[:, 0:1], cstate[:], Tanh)
                    h2 = tmp[:, 1:2]
                    nc.vector.tensor_tensor(h2, ifgo[:, 3:4], tmp[:, 0:1],
                                            op=mybir.AluOpType.mult)
                    # allgather h2 -> full h
                    nc.gpsimd.dma_start(hb_in[:, :], h2)
                    nc.gpsimd.collective_compute(
                        "AllGather", mybir.AluOpType.bypass,
                        replica_groups=groups,
                        ins=[hb_in[:, :].opt()],
                        outs=[hb_out[:, :].opt()],
                    )
                    if step < c.T - 1:
                        nc.sync.dma_start(
                            inpf[:], hb_out[:, 0].rearrange("(c p) -> p c", p=128))
                        nc.vector.tensor_copy(inp[:], inpf[:])
                    else:
                        nc.sync.dma_start(
                            Wf[:], hb_out[:, 0].rearrange("(a b) -> a b", a=c.DIN))
                        nc.vector.tensor_copy(W_sb[:], Wf[:])

            # ---------- phase B: xwd = dinv * (x @ W) ----------
            with (
                tc.tile_pool(name="xw", bufs=3) as xp,
                tc.tile_pool(name="psum_xw", bufs=4, space="PSUM") as ps_xw,
            ):
                xT_sb = xp.tile([c.DIN, c.SHP], BF16, tag="xT")
                nc.sync.dma_start(xT_sb[:], xT_h[:])
                for t in range(c.TILES):
                    pxw = ps_xw.tile([128, c.DH], F32, tag="pxw")
                    nc.tensor.matmul(pxw[:], xT_sb[:, t * 128 : (t + 1) * 128],
                                     W_sb[:], start=True, stop=True)
                    nc.vector.tensor_scalar(
                        xwd_sb[:, t, :], pxw[:], dinv_sb[:, t : t + 1], None,
                        op0=mybir.AluOpType.mult,
                    )
                    nc.sync.dma_start(
                        xwd_own[t * 128 : (t + 1) * 128, 0 : c.DH],
                        xwd_sb[:, t, :],
                    )

            tc.strict_bb_all_engine_barrier()
            nc.gpsimd.collective_compute(
                "AllGather", mybir.AluOpType.bypass,
                replica_groups=groups,
                ins=[xwd_own[:, :].opt()],
                outs=[xwd_full[:, :].opt()],
            )
            tc.strict_bb_all_engine_barrier()

            # ---------- idx planes: replicate 16-row master to 128 partitions ----
            with tc.tile_pool(name="planes", bufs=1) as plp:
                up = plp.tile([128, TOT // 16], I16)
                vp = plp.tile([128, TOT // 16], I16)
                for g in range(8):
                    nc.sync.dma_start(up[16 * g : 16 * (g + 1), :], uidx_h[:, :])
                    nc.sync.dma_start(vp[16 * g : 16 * (g + 1), :], vsidx_h[:, :])

                # ---------- phase 1: gather msgs + scatter-add ----------
                with tc.tile_pool(name="p1", bufs=3) as p1:
                    for bb, coff, clen, plist in struct["pieces"]:
                        msg = p1.tile([128, c.CHUNK // 128, c.ROW], F32, tag="msg")
                        nc.gpsimd.dma_gather(
                            msg[:, : clen // 128, :],
                            xwd_full[bb * 32768 :, :],
                            up[:, coff // 16 : (coff + clen) // 16],
                            clen, clen, c.ROW, single_packet=False,
                        )
                        for po, pl, chain in plist:
                            nc.gpsimd.dma_scatter_add(
                                aggs[chain][:, :],
                                msg[:, po // 128 : (po + pl) // 128, :],
                                vp[:, (coff + po) // 16 : (coff + po + pl) // 16],
                                pl, pl, c.ROW, single_packet=False,
                            )

                tc.strict_bb_all_engine_barrier()

                # ---------- phase 2: emb, uv tables ----------
                with (
                    tc.tile_pool(name="p2", bufs=3) as p2,
                    tc.tile_pool(name="psum_t", bufs=2, space="PSUM") as ps_t,
                    tc.tile_pool(name="psum_uv", bufs=2, space="PSUM") as ps_uv,
                ):
                    for t in range(c.TILES):
                        r0, r1 = t * 128, (t + 1) * 128
                        ag = [p2.tile([128, c.ROW], F32, tag=f"ag{i}", name=f"ag{i}")
                              for i in range(c.CCH)]
                        for i in range(c.CCH):
                            nc.sync.dma_start(ag[i][:], aggs[i][r0:r1, :])
                        s0 = p2.tile([128, c.DH], F32, tag="s0")
                        s1 = p2.tile([128, c.DH], F32, tag="s1")
                        nc.vector.tensor_tensor(s0[:], ag[0][:, : c.DH], ag[1][:, : c.DH],
                                                op=mybir.AluOpType.add)
                        nc.vector.tensor_tensor(s1[:], ag[2][:, : c.DH], ag[3][:, : c.DH],
                                                op=mybir.AluOpType.add)
                        nc.vector.tensor_tensor(s0[:], s0[:], s1[:],
                                                op=mybir.AluOpType.add)
                        nc.vector.tensor_tensor(s0[:], s0[:], xwd_sb[:, t, :],
                                                op=mybir.AluOpType.add)
                        emb = p2.tile([128, c.DH], F32, tag="emb")
                        nc.scalar.activation(emb[:], s0[:],
                                             mybir.ActivationFunctionType.Relu,
                                             scale=dinv_sb[:, t : t + 1])
                        pt = ps_t.tile([c.DH, 128], F32, tag="pt")
                        nc.tensor.transpose(pt[:], emb[:], ident[:])
                        embT = p2.tile([c.DH, 128], F32, tag="embT")
                        nc.vector.tensor_copy(embT[:], pt[:])
                        puv = ps_uv.tile([128, 2 * c.DH], F32, tag="puv")
                        nc.tensor.matmul(puv[:], embT[:], w1ab_sb[:],
                                         start=True, stop=True)
                        uvt = p2.tile([128, c.ROW], F32, tag="uvt")
                        nc.vector.tensor_copy(uvt[:, : 2 * c.DH], puv[:])
                        nc.sync.dma_start(uv_own[r0:r1, :], uvt[:])
                        nc.sync.dma_start(uv_shard[r0:r1, :], uvt[:])

                tc.strict_bb_all_engine_barrier()
                nc.gpsimd.collective_compute(
                    "AllGather", mybir.AluOpType.bypass,
                    replica_groups=groups,
                    ins=[uv_shard[:, :].opt()],
                    outs=[uv_full[:, :].opt()],
                )
                tc.strict_bb_all_engine_barrier()

                # ---------- phase 3: edge MLP ----------
                b2 = struct["b2"]
                with (
                    tc.tile_pool(name="p3", bufs=2) as p3,
                    tc.tile_pool(name="psum_w", bufs=4, space="PSUM") as ps_w,
                ):
                    for bb, coff, clen, _pl in struct["pieces"]:
                        ug = p3.tile([128, c.CHUNK // 128, c.ROW], F32, tag="ug")
                        vg = p3.tile([128, c.CHUNK // 128, c.ROW], F32, tag="vg")
                        nc.gpsimd.dma_gather(
                            ug[:, : clen // 128, :], uv_full[bb * 32768 :, :],
                            up[:, coff // 16 : (coff + clen) // 16],
                            clen, clen, c.ROW, single_packet=False,
                        )
                        nc.gpsimd.dma_gather(
                            vg[:, : clen // 128, :], uv_own[:, :],
                            vp[:, coff // 16 : (coff + clen) // 16],
                            clen, clen, c.ROW, single_packet=False,
                        )
                        eat = p3.tile([NEF, c.CHUNK], BF16, tag="eat")
                        nc.sync.dma_start(eat[:, :clen],
                                          eaT_h[:, coff : coff + clen])
                        lg = p3.tile([128, c.CHUNK // 128], F32, tag="lg")
                        ngrp = -(-clen // 2048)
                        for g in range(ngrp):
                            e0 = g * 2048
                            gl = min(2048, clen - e0)               # multiple of 128
                            nbk = gl // 128
                            pw = ps_w.tile([128, 512], F32, tag="pw")
                            for e in range(nbk):
                                nc.tensor.matmul(
                                    pw[:, e * c.DH : (e + 1) * c.DH],
                                    eat[:, e0 + e * 128 : e0 + (e + 1) * 128],
                                    w1c_sb[:], start=True, stop=True,
                                )
                            z = p3.tile([128, 16, c.DH], F32, tag="z")
                            blk = slice(e0 // 128, e0 // 128 + nbk)
                            nc.vector.tensor_tensor(
                                z[:, :nbk, :], ug[:, blk, : c.DH],
                                vg[:, blk, c.DH : 2 * c.DH], op=mybir.AluOpType.add,
                            )
                            nc.vector.tensor_tensor(
                                z[:].rearrange("p a b -> p (a b)")[:, : nbk * c.DH],
                                z[:].rearrange("p a b -> p (a b)")[:, : nbk * c.DH],
                                pw[:, : nbk * c.DH],
                                op=mybir.AluOpType.add,
                            )
                            nc.scalar.activation(
                                z[:, :nbk, :], z[:, :nbk, :],
                                mybir.ActivationFunctionType.Relu,
                            )
                            nc.vector.tensor_tensor(
                                z[:, :nbk, :], z[:, :nbk, :],
                                w2_sb[:].rearrange("p (a b) -> p a b", b=c.DH)[:, :nbk, :],
                                op=mybir.AluOpType.mult,
                            )
                            nc.vector.tensor_reduce(
                                lg[:, blk], z[:, :nbk, :],
                                axis=mybir.AxisListType.X, op=mybir.AluOpType.add,
                            )
                        if b2 != 0.0:
                            nc.vector.tensor_scalar_add(lg[:, : clen // 128],
                                                        lg[:, : clen // 128], b2)
                        lgb = p3.tile([128, c.CHUNK // 128], BF16, tag="lgb")
                        nc.vector.tensor_copy(lgb[:, : clen // 128],
                                              lg[:, : clen // 128])
                        nc.sync.dma_start(
                            logits_h[:, coff // 128 : (coff + clen) // 128],
                            lgb[:, : clen // 128],
                        )

    nc.compile()
    return nc


_BUILD_CACHE = {}


def _kernel_impl(inputs, cfg):
    in_maps, struct, orig_all = host_prep(inputs, cfg)
    key = (cfg.N, cfg.E, struct["TOT"], struct["NEF"], str(struct["pieces"]),
           struct["b2"])
    if key not in _BUILD_CACHE:
        _BUILD_CACHE.clear()
        _BUILD_CACHE[key] = build(cfg, struct)
    nc = _BUILD_CACHE[key]
    res = run_bass_kernel_spmd(nc, in_maps, list(range(cfg.NC)))
    out = np.empty(cfg.E, np.float32)
    flat = np.concatenate([res.results[k]["logits"].astype(np.float32).T.reshape(-1)
                           for k in range(cfg.NC)])
    valid = orig_all >= 0
    out[orig_all[valid]] = flat[valid]
    return out


def kernel(**inputs):
    cfg = CFG(N=100000, E=1_600_000, T=5, DIN=32, DH=32, EF=16)
    return _kernel_impl(inputs, cfg)


# revision 39
# speedup vs baseline: 1.2278x; 1.2278x over previous
"""Trainium2 Bass kernel for nn_EvolvingGNN (LSTM-evolved GCN + edge MLP).

Strategy (8 NeuronCores, full inputs in / full output out):
  - Nodes sharded 12500/core. Edges partitioned by destination core.
  - LSTM distributed: each core computes 512 of the 4096 gate rows
    (reads only its slice of W_ih/W_hh), AllGather of h each step.
  - xwd[n] = dinv[n] * (x[n] @ W) computed on the node shard, AllGathered
    into a full 256B-row table for gathers.
  - Message phase: dma_gather xwd[src] -> dma_scatter_add into agg[dst]
    (CCE add). Scatter calls must have unique indices (duplicate rows in
    one call race on read-modify-write), so edges are organised into
    "rounds" (r-th in-edge of each node) with round-robin over 4
    accumulator tables to hide the inter-round ordering latency.
  - emb = relu(dinv * (agg + xwd_self)); uv = [emb@W1a.T | emb@W1b.T]
    (one 256B row per node), AllGathered.
  - Edge MLP: gather uv[src] (u half) + uv[dst] (v half), w = ea@W1c.T+b1
    via PE matmuls on host-transposed edge_attr, logits = relu(z) . W2 + b2
    via DVE mul+reduce.
  - Gather indices are int16, so the node-table rows are bucketed in
    32768-row groups; the per-core edge order is (bucket, round, dst).
    Pads: gathers use row 0, scatters use a trash row.

Wall-clock optimisations (the axon tunnel moves ~60MB/s, so host->device
bytes dominate the end-to-end time):
  - Accumulator tables and uv_own are Internal DRAM zeroed on device
    (previously ExternalOutputs: ~13MB/core of donated zeros uploaded and
    ~16MB/core of unused outputs downloaded per call).
  - Big payloads (edge features, x, LSTM weights) travel as bfloat16;
    matmuls run bf16 x bf16 -> f32 PSUM.
  - Gather/scatter index planes are sent as the 16-partition master copy
    and replicated to the 128-partition layout on device (8x fewer bytes).
  - host_prep is vectorised: one combined-key argsort pipeline over all
    edges instead of per-core lexsorts.
"""

import os
import pickle
import subprocess
import sys

import numpy as np
import ml_dtypes

import concourse.bacc as bacc
import concourse.mybir as mybir
import concourse.tile as tile
from concourse.bass_utils import run_bass_kernel_spmd
from concourse.masks import make_identity

F32 = mybir.dt.float32
BF16 = mybir.dt.bfloat16
I16 = mybir.dt.int16
NPBF16 = ml_dtypes.bfloat16


class CFG:
    def __init__(self, N, E, T, DIN, DH, EF, NC=8, CHUNK=8192, CCH=4):
        self.N, self.E, self.T = N, E, T
        self.DIN, self.DH, self.EF = DIN, DH, EF
        self.FLAT = DIN * DH
        self.NC = NC
        assert N % NC == 0
        self.SH = N // NC                       # nodes per core
        self.TILES = -(-self.SH // 128)         # node tiles per core
        self.SHP = self.TILES * 128             # padded shard rows
        self.NTAB = NC * self.SHP               # full table rows
        self.NBUCK = -(-self.NTAB // 32768)
        self.CHUNK = CHUNK                      # gather chunk (edges)
        self.CCH = CCH                          # scatter chain tables
        self.ROW = 64                           # table row f32 (256B)
        # LSTM slicing: core k owns gate rows {g*FLAT + k*GSL + j}
        assert (4 * self.FLAT) % NC == 0
        self.GSL = self.FLAT // NC              # per-gate slice (128)
        self.KCH = self.FLAT // 128             # contraction chunks (8)


def _roundup(x, m):
    return -(-x // m) * m


# ---------------------------------------------------------------------------
# Parallel host prep: 8 numpy-only worker subprocesses over shared memory.
# Phase A: per-core edge sort into (bucket, round, dst) order + round counts.
# Phase B: slot assignment + assembly of idx planes / edge features / weights.
# ---------------------------------------------------------------------------

_WORKER_SRC = r"""
import sys, pickle
import numpy as np
import ml_dtypes
from multiprocessing import shared_memory

BF16 = ml_dtypes.bfloat16
_inp = sys.stdin.buffer
_out = sys.stdout.buffer
_shm = {}
_state = {}


def att(name):
    s = _shm.get(name)
    if s is None:
        s = shared_memory.SharedMemory(name=name, track=False)
        _shm[name] = s
    return s


def view(name, shape, dtype):
    n = int(np.prod(shape)) * np.dtype(dtype).itemsize
    return np.ndarray(shape, dtype, buffer=att(name).buf[:n])


while True:
    try:
        cmd = pickle.load(_inp)
    except EOFError:
        break
    op = cmd["op"]
    if op == "A":
        k = cmd["k"]; E = cmd["E"]; SH = cmd["SH"]; SHP = cmd["SHP"]
        NBUCK = cmd["NBUCK"]
        ei = view(cmd["ei"], (2, E), np.int32)
        dst = ei[1]
        lo = k * SH
        eids = np.flatnonzero((dst >= lo) & (dst < lo + SH))
        n = len(eids)
        s = ei[0][eids].astype(np.int64)
        rowid = (s // SH) * SHP + (s % SH)
        sbuck = (rowid >> 15).astype(np.int32)
        s16 = (rowid & 32767).astype(np.int16)
        dloc = (dst[eids] - lo).astype(np.int32)
        o1 = np.argsort(sbuck * np.int32(SH) + dloc, kind="stable")
        b1, d1 = sbuck[o1], dloc[o1]
        k1 = b1 * np.int32(SH) + d1
        newrun = np.empty(n, bool); newrun[:1] = True
        np.not_equal(k1[1:], k1[:-1], out=newrun[1:])
        starts = np.flatnonzero(newrun)
        r1 = (np.arange(n) - np.repeat(starts, np.diff(np.r_[starts, n]))).astype(np.int32)
        MAXR = int(r1.max()) + 1 if n else 1
        o2 = np.argsort((b1 * np.int32(MAXR) + r1) * np.int32(SH) + d1,
                        kind="stable")
        _state["eids"] = eids[o1][o2]
        _state["b"] = b1[o2]
        _state["r"] = r1[o2]
        _state["d"] = d1[o2]
        _state["s16"] = s16[o1][o2]
        _state["MAXR"] = MAXR
        cnt = np.bincount(_state["b"] * np.int32(MAXR) + _state["r"],
                          minlength=NBUCK * MAXR).reshape(NBUCK, MAXR)
        pickle.dump(cnt, _out); _out.flush()
    elif op == "B":
        k = cmd["k"]; E = cmd["E"]; TOT = cmd["TOT"]; EF = cmd["EF"]
        NEF = cmd["NEF"]; SHP = cmd["SHP"]
        seg_off = cmd["seg_off"]                       # [NBUCK, MAXR_glob]
        b, r, d = _state["b"], _state["r"], _state["d"]
        eids, s16, MAXR = _state["eids"], _state["s16"], _state["MAXR"]
        n = len(eids)
        ckey = b * np.int32(MAXR) + r
        newseg = np.empty(n, bool); newseg[:1] = True
        np.not_equal(ckey[1:], ckey[:-1], out=newseg[1:])
        sstarts = np.flatnonzero(newseg)
        rank = np.arange(n) - np.repeat(sstarts, np.diff(np.r_[sstarts, n]))
        slot = seg_off[b, r] + rank
        planes = view(cmd["planes"], (8, 2, 16, TOT // 16), np.int16)
        eaT = view(cmd["eaT"], (8, NEF, TOT), BF16)
        orig = view(cmd["orig"], (8, TOT), np.int32)
        ea = view(cmd["ea"], (E, EF), np.float32)
        u16 = np.zeros(TOT, np.int16); u16[slot] = s16
        vs = np.full(TOT, SHP, np.int16); vs[slot] = d.astype(np.int16)
        planes[k, 0] = u16.reshape(TOT // 16, 16).T
        planes[k, 1] = vs.reshape(TOT // 16, 16).T
        og = np.full(TOT, -1, np.int32); og[slot] = eids.astype(np.int32)
        orig[k] = og
        rows = np.zeros((TOT, NEF), BF16)
        rows[slot, : EF] = ea[eids].astype(BF16)
        if NEF > EF:
            rows[slot, EF] = 1.0
        eaT[k] = rows.T
        pickle.dump(k, _out); _out.flush()
"""


class _PrepPool:
    def __init__(self, n=8):
        self.n = n
        self.procs = [
            subprocess.Popen([sys.executable, "-u", "-c", _WORKER_SRC],
                             stdin=subprocess.PIPE, stdout=subprocess.PIPE)
            for _ in range(n)
        ]
        self.shms = {}

    def arr(self, key, shape, dtype):
        from multiprocessing import shared_memory
        nbytes = int(np.prod(shape)) * np.dtype(dtype).itemsize
        cur = self.shms.get(key)
        if cur is None or cur.size < nbytes:
            if cur is not None:
                try:
                    cur.close(); cur.unlink()
                except Exception:
                    pass
            cur = shared_memory.SharedMemory(create=True, size=nbytes)
            self.shms[key] = cur
        return np.ndarray(shape, dtype, buffer=cur.buf[:nbytes]), cur.name

    def send(self, i, obj):
        pickle.dump(obj, self.procs[i].stdin)
        self.procs[i].stdin.flush()

    def recv(self, i):
        return pickle.load(self.procs[i].stdout)

    def kill(self):
        for p in self.procs:
            try:
                p.kill()
            except Exception:
                pass
        for s in self.shms.values():
            try:
                s.close(); s.unlink()
            except Exception:
                pass


_POOL = None


def _get_pool():
    global _POOL
    if _POOL is None:
        _POOL = _PrepPool()
    return _POOL


def _host_prep_parallel(inputs, cfg):
    c = cfg
    pool = _get_pool()
    ei = np.asarray(inputs["edge_index"])
    ei_shm, ei_name = pool.arr("ei", (2, c.E), np.int32)
    np.copyto(ei_shm, ei)
    ea_shm, ea_name = pool.arr("ea", (c.E, c.EF), np.float32)
    np.copyto(ea_shm, np.asarray(inputs["edge_attr"], np.float32))
    for k in range(c.NC):
        pool.send(k, {"op": "A", "k": k, "E": c.E, "SH": c.SH, "SHP": c.SHP,
                      "NBUCK": c.NBUCK, "ei": ei_name})

    # parent-side smalls while workers sort
    wmat = _host_lstm(inputs, cfg)
    x_last = np.asarray(inputs["x"][-1], np.float32)
    xlT16 = x_last.T.astype(NPBF16)                             # [DIN, N]
    dst = ei_shm[1]
    deg = np.bincount(dst, minlength=c.N).astype(np.float32) + 1.0
    dinv = (1.0 / np.sqrt(deg)).astype(np.float32)
    W1 = np.asarray(inputs["W1"], np.float32)
    w1ab = np.ascontiguousarray(
        np.concatenate([W1[:, : c.DH].T, W1[:, c.DH : 2 * c.DH].T], axis=1))
    b1v = np.asarray(inputs["b1"], np.float32)
    has_b1 = bool(np.any(b1v))
    NEF = c.EF + 1 if has_b1 else c.EF
    w1c_parts = [W1[:, 2 * c.DH :].T] + ([b1v[None, :]] if has_b1 else [])
    w1c = np.ascontiguousarray(np.concatenate(w1c_parts).astype(NPBF16))
    w2 = np.asarray(inputs["W2"], np.float32).reshape(-1)
    w2row = np.ascontiguousarray(np.tile(w2, 512 // c.DH)[None, :])

    cnts = [pool.recv(k) for k in range(c.NC)]
    MAXR = max(cn.shape[1] for cn in cnts)
    segmax = np.zeros((c.NBUCK, MAXR), np.int64)
    for cn in cnts:
        np.maximum(segmax[:, : cn.shape[1]], cn, out=segmax[:, : cn.shape[1]])
    segsz = np.where(segmax > 0, ((segmax + 127) // 128) * 128, 0).astype(np.int64)
    seg_off = np.concatenate([[0], np.cumsum(segsz.reshape(-1))])[:-1].reshape(
        c.NBUCK, MAXR)
    TOT = int(segsz.sum())

    blen = segsz.sum(axis=1)
    bstarts = np.concatenate([[0], np.cumsum(blen)])
    pieces = []
    piece_ctr = 0
    for bb in range(c.NBUCK):
        bstart, bl = int(bstarts[bb]), int(blen[bb])
        if bl == 0:
            continue
        cuts = list(range(bstart, bstart + bl, c.CHUNK)) + [bstart + bl]
        for ci in range(len(cuts) - 1):
            coff, cend = cuts[ci], cuts[ci + 1]
            plist = []
            for rv in range(MAXR):
                if segsz[bb, rv] == 0:
                    continue
                so = int(seg_off[bb, rv])
                se = so + int(segsz[bb, rv])
                lo, hi = max(so, coff), min(se, cend)
                while lo < hi:
                    sub = min(hi - lo, 4096)
                    plist.append((lo - coff, sub, piece_ctr % c.CCH))
                    piece_ctr += 1
                    lo += sub
            pieces.append((bb, coff, cend - coff, plist))

    planes_shm, planes_name = pool.arr("planes", (8, 2, 16, TOT // 16), np.int16)
    eaT_shm, eaT_name = pool.arr("eaTo", (8, NEF, TOT), NPBF16)
    orig_shm, orig_name = pool.arr("orig", (8, TOT), np.int32)
    for k in range(c.NC):
        pool.send(k, {"op": "B", "k": k, "E": c.E, "TOT": TOT, "EF": c.EF,
                      "NEF": NEF, "SHP": c.SHP,
                      "seg_off": seg_off, "planes": planes_name,
                      "eaT": eaT_name, "orig": orig_name, "ea": ea_name})

    in_maps = []
    for k in range(c.NC):
        n0 = k * c.SH
        xT = np.zeros((c.DIN, c.SHP), NPBF16)
        xT[:, : c.SH] = xlT16[:, n0 : n0 + c.SH]
        dflat = np.ones(c.SHP, np.float32)
        dflat[: c.SH] = dinv[n0 : n0 + c.SH]
        dvt = np.ascontiguousarray(dflat.reshape(c.TILES, 128).T)
        in_maps.append({
            "xT": xT, "dinv": dvt, "wmat": wmat,
            "w1ab": w1ab, "w1c": w1c, "w2row": w2row,
            "uidx": planes_shm[k, 0], "vsidx": planes_shm[k, 1],
            "eaT": eaT_shm[k],
        })
    for k in range(c.NC):
        pool.recv(k)

    struct = {
        "TOT": TOT,
        "NEF": NEF,
        "pieces": pieces,
        "b2": float(np.asarray(inputs["b2"], np.float32).reshape(-1)[0]),
    }
    return in_maps, struct, orig_shm.reshape(-1)


def _ncpu():
    try:
        return len(os.sched_getaffinity(0))
    except Exception:
        return os.cpu_count() or 1


def _host_lstm(inputs, cfg):
    """The weight-evolving LSTM depends only on (tiny) host-known inputs —
    42 MFLOP of serial matvecs. Run it on host in f32 (exact vs reference)
    instead of shipping 33MB of LSTM weights through the slow tunnel."""
    c = cfg
    W_ih = np.asarray(inputs["W_ih"], np.float32)
    W_hh = np.asarray(inputs["W_hh"], np.float32)
    b = (np.asarray(inputs["b_ih"], np.float32)
         + np.asarray(inputs["b_hh"], np.float32))
    inp = np.asarray(inputs["initial_weights"], np.float32).reshape(-1)
    h = np.zeros(c.FLAT, np.float32)
    cs = np.zeros(c.FLAT, np.float32)
    for _ in range(c.T):
        gates = W_ih @ inp + W_hh @ h + b
        i, f, g, o = np.split(gates, 4)
        i = 1.0 / (1.0 + np.exp(-i))
        f = 1.0 / (1.0 + np.exp(-f))
        g = np.tanh(g)
        o = 1.0 / (1.0 + np.exp(-o))
        cs = f * cs + i * g
        h = o * np.tanh(cs)
        inp = h
    return np.ascontiguousarray(h.reshape(c.DIN, c.DH).astype(NPBF16))


def host_prep(inputs, cfg):
    # The worker pool only pays off with real parallelism; on the 1-2 CPU
    # containers the serial vectorised path is strictly better.
    global _POOL
    if os.environ.get("KPREP_SERIAL") != "1" and _ncpu() >= 4:
        try:
            return _host_prep_parallel(inputs, cfg)
        except Exception:
            if _POOL is not None:
                _POOL.kill()
                _POOL = None
    return _host_prep_serial(inputs, cfg)


def _host_prep_serial(inputs, cfg):
    """Shard / reorder everything on the host. Returns (in_maps, struct, origs)."""
    c = cfg
    x_last = np.asarray(inputs["x"][-1], np.float32)            # [N, DIN]
    ei = np.asarray(inputs["edge_index"])                       # [2, E]
    ea = np.asarray(inputs["edge_attr"], np.float32)            # [E, EF]
    src = ei[0].astype(np.int32)
    dst = ei[1].astype(np.int32)

    deg = np.bincount(dst, minlength=c.N).astype(np.float32) + 1.0
    dinv = (1.0 / np.sqrt(deg)).astype(np.float32)

    rowid = (src // c.SH) * c.SHP + (src % c.SH)                # table row of src
    sbuck = rowid >> 15
    s16 = (rowid & 32767).astype(np.int16)
    ecore = dst // c.SH
    dloc = dst - ecore * c.SH

    # ---- global (core, bucket, round, dst) ordering ----
    key1 = (ecore * c.NBUCK + sbuck) * c.SH + dloc              # int32
    o1 = np.argsort(key1, kind="stable").astype(np.int32)
    k1 = key1[o1]
    newrun = np.empty(c.E, bool)
    newrun[0] = True
    np.not_equal(k1[1:], k1[:-1], out=newrun[1:])
    starts = np.flatnonzero(newrun).astype(np.int32)
    ar = np.arange(c.E, dtype=np.int32)
    r1 = ar - np.repeat(starts, np.diff(np.r_[starts, np.int32(c.E)]))
    MAXR = int(r1.max()) + 1
    key2 = ((ecore[o1] * c.NBUCK + sbuck[o1]) * np.int32(MAXR) + r1) * c.SH \
        + dloc[o1]
    o2 = np.argsort(key2, kind="stable").astype(np.int32)
    eid2 = o1[o2]
    ec2, b2v, r2, d2 = ecore[eid2], sbuck[eid2], r1[o2], dloc[eid2]

    # ---- universal segment sizes: max count over cores per (bucket, round) ----
    ckey = (ec2 * c.NBUCK + b2v) * np.int32(MAXR) + r2
    cnt = np.bincount(ckey, minlength=c.NC * c.NBUCK * MAXR).reshape(
        c.NC, c.NBUCK, MAXR)
    segmax = cnt.max(axis=0)                                    # [NBUCK, MAXR]
    segsz = np.where(segmax > 0, ((segmax + 127) // 128) * 128, 0).astype(np.int64)
    seg_off = np.concatenate([[0], np.cumsum(segsz.reshape(-1))])[:-1].reshape(
        c.NBUCK, MAXR).astype(np.int32)
    TOT = int(segsz.sum())
    assert TOT % 128 == 0

    # ---- per-edge slot ----
    newseg = np.empty(c.E, bool)
    newseg[0] = True
    np.not_equal(ckey[1:], ckey[:-1], out=newseg[1:])
    sstarts = np.flatnonzero(newseg).astype(np.int32)
    rank = ar - np.repeat(sstarts, np.diff(np.r_[sstarts, np.int32(c.E)]))
    slot = seg_off[b2v, r2] + rank                              # [0, TOT) per core
    gslot = ec2 * np.int32(TOT) + slot

    # ---- chunk / scatter-piece structure (identical for all cores) ----
    blen = segsz.sum(axis=1)                                    # per bucket
    bstarts = np.concatenate([[0], np.cumsum(blen)])
    pieces = []                                                 # (bb,coff,clen,[(po,pl,chain)])
    piece_ctr = 0
    for bb in range(c.NBUCK):
        bstart, bl = int(bstarts[bb]), int(blen[bb])
        if bl == 0:
            continue
        cuts = list(range(bstart, bstart + bl, c.CHUNK)) + [bstart + bl]
        for ci in range(len(cuts) - 1):
            coff, cend = cuts[ci], cuts[ci + 1]
            plist = []
            for rv in range(MAXR):
                if segsz[bb, rv] == 0:
                    continue
                so = int(seg_off[bb, rv])
                se = so + int(segsz[bb, rv])
                lo, hi = max(so, coff), min(se, cend)
                # dma_scatter_add breaks above 4096 idxs per call
                while lo < hi:
                    sub = min(hi - lo, 4096)
                    plist.append((lo - coff, sub, piece_ctr % c.CCH))
                    piece_ctr += 1
                    lo += sub
            pieces.append((bb, coff, cend - coff, plist))

    # ---- global slot-order tables ----
    TRASH = c.SHP                                               # scatter/v pad row
    NT = c.NC * TOT
    u16_all = np.zeros(NT, np.int16)
    u16_all[gslot] = s16[eid2]
    vs_all = np.full(NT, TRASH, np.int16)
    vs_all[gslot] = d2.astype(np.int16)
    orig_all = np.full(NT, -1, np.int32)
    orig_all[gslot] = eid2

    b1v = np.asarray(inputs["b1"], np.float32)
    has_b1 = bool(np.any(b1v))
    NEF = c.EF + 1 if has_b1 else c.EF
    ea16 = ea.astype(NPBF16)
    ea_rows = np.zeros((NT, NEF), NPBF16)
    ea_rows[gslot, : c.EF] = ea16[eid2]
    if has_b1:
        ea_rows[gslot, c.EF] = 1.0

    xlT16 = x_last.T.astype(NPBF16)                             # [DIN, N]

    W1 = np.asarray(inputs["W1"], np.float32)                   # [DH, 2DH+EF]
    w1ab = np.ascontiguousarray(
        np.concatenate([W1[:, : c.DH].T, W1[:, c.DH : 2 * c.DH].T], axis=1))
    w1c_parts = [W1[:, 2 * c.DH :].T] + ([b1v[None, :]] if has_b1 else [])
    w1c = np.ascontiguousarray(np.concatenate(w1c_parts).astype(NPBF16))
    w2 = np.asarray(inputs["W2"], np.float32).reshape(-1)       # [DH]
    w2row = np.ascontiguousarray(np.tile(w2, 512 // c.DH)[None, :])  # [1, 512]
    wmat = _host_lstm(inputs, cfg)                              # [DIN, DH] bf16

    in_maps = []
    for k in range(c.NC):
        sl = slice(k * TOT, (k + 1) * TOT)
        n0 = k * c.SH

        xT = np.zeros((c.DIN, c.SHP), NPBF16)
        xT[:, : c.SH] = xlT16[:, n0 : n0 + c.SH]
        dflat = np.ones(c.SHP, np.float32)
        dflat[: c.SH] = dinv[n0 : n0 + c.SH]
        dvt = np.ascontiguousarray(dflat.reshape(c.TILES, 128).T)

        in_maps.append({
            "xT": xT,
            "dinv": dvt,
            "wmat": wmat,
            "w1ab": w1ab,
            "w1c": w1c,
            "w2row": w2row,
            "uidx": np.ascontiguousarray(u16_all[sl].reshape(TOT // 16, 16).T),
            "vsidx": np.ascontiguousarray(vs_all[sl].reshape(TOT // 16, 16).T),
            "eaT": np.ascontiguousarray(ea_rows[sl].T),         # [NEF, TOT] bf16
        })

    struct = {
        "TOT": TOT,
        "NEF": NEF,
        "pieces": pieces,
        "b2": float(np.asarray(inputs["b2"], np.float32).reshape(-1)[0]),
    }
    return in_maps, struct, orig_all


def build(cfg, struct):
    c = cfg
    TOT = struct["TOT"]
    NEF = struct["NEF"]
    nc = bacc.Bacc("TRN2", target_bir_lowering=False, debug=False,
                   num_devices=c.NC)

    # ---------- I/O ----------
    xT_h = nc.dram_tensor("xT", [c.DIN, c.SHP], BF16, kind="ExternalInput")
    dinv_h = nc.dram_tensor("dinv", [128, c.TILES], F32, kind="ExternalInput")
    wmat_h = nc.dram_tensor("wmat", [c.DIN, c.DH], BF16, kind="ExternalInput")
    w1ab_h = nc.dram_tensor("w1ab", [c.DH, 2 * c.DH], F32, kind="ExternalInput")
    w1c_h = nc.dram_tensor("w1c", [NEF, c.DH], BF16, kind="ExternalInput")
    w2row_h = nc.dram_tensor("w2row", [1, 512], F32, kind="ExternalInput")
    uidx_h = nc.dram_tensor("uidx", [16, TOT // 16], I16, kind="ExternalInput")
    vsidx_h = nc.dram_tensor("vsidx", [16, TOT // 16], I16, kind="ExternalInput")
    eaT_h = nc.dram_tensor("eaT", [NEF, TOT], BF16, kind="ExternalInput")

    logits_h = nc.dram_tensor("logits", [128, TOT // 128], BF16, kind="ExternalOutput")
    # internal accumulator tables, zeroed on device before the scatter phase
    aggs = [nc.dram_tensor(f"agg{i}", [c.SHP + 128, c.ROW], F32)
            for i in range(c.CCH)]
    uv_own = nc.dram_tensor("uv_own", [c.SHP + 128, c.ROW], F32)

    # internal DRAM
    xwd_own = nc.dram_tensor("xwd_own", [c.SHP, c.ROW], F32)
    xwd_full = nc.dram_tensor("xwd_full", [c.NTAB, c.ROW], F32, addr_space="Shared")
    uv_shard = nc.dram_tensor("uv_shard", [c.SHP, c.ROW], F32)
    uv_full = nc.dram_tensor("uv_full", [c.NTAB, c.ROW], F32, addr_space="Shared")

    groups = [list(range(c.NC))]

    with tile.TileContext(nc) as tc:
        with (
            tc.tile_pool(name="persist", bufs=1) as pp,
            tc.tile_pool(name="psum_ls", bufs=2, space="PSUM") as ps_ls,
        ):
            # ---------- persistent small tiles ----------
            ident = pp.tile([128, 128], F32)
            make_identity(nc, ident[:])
            w1ab_sb = pp.tile([c.DH, 2 * c.DH], F32)
            nc.sync.dma_start(w1ab_sb[:], w1ab_h[:])
            w1c_sb = pp.tile([NEF, c.DH], BF16)
            nc.sync.dma_start(w1c_sb[:], w1c_h[:])
            dinv_sb = pp.tile([128, c.TILES], F32)
            nc.sync.dma_start(dinv_sb[:], dinv_h[:])
            xwd_sb = pp.tile([128, c.TILES, c.DH], F32)  # persists to post-agg
            W_sb = pp.tile([c.DIN, c.DH], BF16)          # evolved GCN weight
            nc.sync.dma_start(W_sb[:], wmat_h[:])

            # w2 broadcast [1,512] -> [128,512] via K=1 matmul with ones
            w2r_sb = pp.tile([1, 512], F32)
            nc.sync.dma_start(w2r_sb[:], w2row_h[:])
            ones1 = pp.tile([1, 128], F32)
            nc.vector.memset(ones1[:], 1.0)
            w2_sb = pp.tile([128, 512], F32)
            pw2 = ps_ls.tile([128, 512], F32, tag="w2bc")
            nc.tensor.matmul(pw2[:], ones1[:], w2r_sb[:], start=True, stop=True)
            nc.vector.tensor_copy(w2_sb[:], pw2[:])

            # ---------- zero the accumulator tables (device-side) ----------
            zt = pp.tile([128, 16, c.ROW], F32)
            nc.vector.memset(zt[:], 0.0)
            ntile = (c.SHP + 128) // 128
            for t in aggs:
                av = t[:, :].rearrange("(x p) c -> p x c", p=128)
                for x0 in range(0, ntile, 16):
                    xl = min(16, ntile - x0)
                    nc.sync.dma_start(av[:, x0 : x0 + xl, :], zt[:, :xl, :])
            nc.sync.dma_start(uv_own[c.SHP : c.SHP + 128, :], zt[:, 0, :])

            # ---------- phase B: xwd = dinv * (x @ W) ----------
            with (
                tc.tile_pool(name="xw", bufs=3) as xp,
                tc.tile_pool(name="psum_xw", bufs=4, space="PSUM") as ps_xw,
            ):
                xT_sb = xp.tile([c.DIN, c.SHP], BF16, tag="xT")
                nc.sync.dma_start(xT_sb[:], xT_h[:])
                for t in range(c.TILES):
                    pxw = ps_xw.tile([128, c.DH], F32, tag="pxw")
                    nc.tensor.matmul(pxw[:], xT_sb[:, t * 128 : (t + 1) * 128],
                                     W_sb[:], start=True, stop=True)
                    nc.vector.tensor_scalar(
                        xwd_sb[:, t, :], pxw[:], dinv_sb[:, t : t + 1], None,
                        op0=mybir.AluOpType.mult,
                    )
                    nc.sync.dma_start(
                        xwd_own[t * 128 : (t + 1) * 128, 0 : c.DH],
                        xwd_sb[:, t, :],
                    )

            tc.strict_bb_all_engine_barrier()
            nc.gpsimd.collective_compute(
                "AllGather", mybir.AluOpType.bypass,
                replica_groups=groups,
                ins=[xwd_own[:, :].opt()],
                outs=[xwd_full[:, :].opt()],
            )
            tc.strict_bb_all_engine_barrier()

            # ---------- idx planes: replicate 16-row master to 128 partitions ----
            with tc.tile_pool(name="planes", bufs=1) as plp:
                up = plp.tile([128, TOT // 16], I16)
                vp = plp.tile([128, TOT // 16], I16)
                for g in range(8):
                    nc.sync.dma_start(up[16 * g : 16 * (g + 1), :], uidx_h[:, :])
                    nc.sync.dma_start(vp[16 * g : 16 * (g + 1), :], vsidx_h[:, :])

                # ---------- phase 1: gather msgs + scatter-add ----------
                with tc.tile_pool(name="p1", bufs=3) as p1:
                    for bb, coff, clen, plist in struct["pieces"]:
                        msg = p1.tile([128, c.CHUNK // 128, c.ROW], F32, tag="msg")
                        nc.gpsimd.dma_gather(
                            msg[:, : clen // 128, :],
                            xwd_full[bb * 32768 :, :],
                            up[:, coff // 16 : (coff + clen) // 16],
                            clen, clen, c.ROW, single_packet=False,
                        )
                        for po, pl, chain in plist:
                            nc.gpsimd.dma_scatter_add(
                                aggs[chain][:, :],
                                msg[:, po // 128 : (po + pl) // 128, :],
                                vp[:, (coff + po) // 16 : (coff + po + pl) // 16],
                                pl, pl, c.ROW, single_packet=False,
                            )

                tc.strict_bb_all_engine_barrier()

                # ---------- phase 2: emb, uv tables ----------
                with (
                    tc.tile_pool(name="p2", bufs=3) as p2,
                    tc.tile_pool(name="psum_t", bufs=2, space="PSUM") as ps_t,
                    tc.tile_pool(name="psum_uv", bufs=2, space="PSUM") as ps_uv,
                ):
                    for t in range(c.TILES):
                        r0, r1 = t * 128, (t + 1) * 128
                        ag = [p2.tile([128, c.ROW], F32, tag=f"ag{i}", name=f"ag{i}")
                              for i in range(c.CCH)]
                        for i in range(c.CCH):
                            nc.sync.dma_start(ag[i][:], aggs[i][r0:r1, :])
                        s0 = p2.tile([128, c.DH], F32, tag="s0")
                        s1 = p2.tile([128, c.DH], F32, tag="s1")
                        nc.vector.tensor_tensor(s0[:], ag[0][:, : c.DH], ag[1][:, : c.DH],
                                                op=mybir.AluOpType.add)
                        nc.vector.tensor_tensor(s1[:], ag[2][:, : c.DH], ag[3][:, : c.DH],
                                                op=mybir.AluOpType.add)
                        nc.vector.tensor_tensor(s0[:], s0[:], s1[:],
                                                op=mybir.AluOpType.add)
                        nc.vector.tensor_tensor(s0[:], s0[:], xwd_sb[:, t, :],
                                                op=mybir.AluOpType.add)
                        emb = p2.tile([128, c.DH], F32, tag="emb")
                        nc.scalar.activation(emb[:], s0[:],
                                             mybir.ActivationFunctionType.Relu,
                                             scale=dinv_sb[:, t : t + 1])
                        pt = ps_t.tile([c.DH, 128], F32, tag="pt")
                        nc.tensor.transpose(pt[:], emb[:], ident[:])
                        embT = p2.tile([c.DH, 128], F32, tag="embT")
                        nc.vector.tensor_copy(embT[:], pt[:])
                        puv = ps_uv.tile([128, 2 * c.DH], F32, tag="puv")
                        nc.tensor.matmul(puv[:], embT[:], w1ab_sb[:],
                                         start=True, stop=True)
                        uvt = p2.tile([128, c.ROW], F32, tag="uvt")
                        nc.vector.tensor_copy(uvt[:, : 2 * c.DH], puv[:])
                        nc.sync.dma_start(uv_own[r0:r1, :], uvt[:])
                        nc.sync.dma_start(uv_shard[r0:r1, :], uvt[:])

                tc.strict_bb_all_engine_barrier()
                nc.gpsimd.collective_compute(
                    "AllGather", mybir.AluOpType.bypass,
                    replica_groups=groups,
                    ins=[uv_shard[:, :].opt()],
                    outs=[uv_full[:, :].opt()],
                )
                tc.strict_bb_all_engine_barrier()

                # ---------- phase 3: edge MLP ----------
                b2 = struct["b2"]
                with (
                    tc.tile_pool(name="p3", bufs=2) as p3,
                    tc.tile_pool(name="psum_w", bufs=4, space="PSUM") as ps_w,
                ):
                    for bb, coff, clen, _pl in struct["pieces"]:
                        ug = p3.tile([128, c.CHUNK // 128, c.ROW], F32, tag="ug")
                        vg = p3.tile([128, c.CHUNK // 128, c.ROW], F32, tag="vg")
                        nc.gpsimd.dma_gather(
                            ug[:, : clen // 128, :], uv_full[bb * 32768 :, :],
                            up[:, coff // 16 : (coff + clen) // 16],
                            clen, clen, c.ROW, single_packet=False,
                        )
                        nc.gpsimd.dma_gather(
                            vg[:, : clen // 128, :], uv_own[:, :],
                            vp[:, coff // 16 : (coff + clen) // 16],
                            clen, clen, c.ROW, single_packet=False,
                        )
                        eat = p3.tile([NEF, c.CHUNK], BF16, tag="eat")
                        nc.sync.dma_start(eat[:, :clen],
                                          eaT_h[:, coff : coff + clen])
                        lg = p3.tile([128, c.CHUNK // 128], F32, tag="lg")
                        ngrp = -(-clen // 2048)
                        for g in range(ngrp):
                            e0 = g * 2048
                            gl = min(2048, clen - e0)               # multiple of 128
                            nbk = gl // 128
                            pw = ps_w.tile([128, 512], F32, tag="pw")
                            for e in range(nbk):
                                nc.tensor.matmul(
                                    pw[:, e * c.DH : (e + 1) * c.DH],
                                    eat[:, e0 + e * 128 : e0 + (e + 1) * 128],
                                    w1c_sb[:], start=True, stop=True,
                                )
                            z = p3.tile([128, 16, c.DH], F32, tag="z")
                            blk = slice(e0 // 128, e0 // 128 + nbk)
                            nc.vector.tensor_tensor(
                                z[:, :nbk, :], ug[:, blk, : c.DH],
                                vg[:, blk, c.DH : 2 * c.DH], op=mybir.AluOpType.add,
                            )
                            nc.vector.tensor_tensor(
                                z[:].rearrange("p a b -> p (a b)")[:, : nbk * c.DH],
                                z[:].rearrange("p a b -> p (a b)")[:, : nbk * c.DH],
                                pw[:, : nbk * c.DH],
                                op=mybir.AluOpType.add,
                            )
                            nc.scalar.activation(
                                z[:, :nbk, :], z[:, :nbk, :],
                                mybir.ActivationFunctionType.Relu,
                            )
                            nc.vector.tensor_tensor(
                                z[:, :nbk, :], z[:, :nbk, :],
                                w2_sb[:].rearrange("p (a b) -> p a b", b=c.DH)[:, :nbk, :],
                                op=mybir.AluOpType.mult,
                            )
                            nc.vector.tensor_reduce(
                                lg[:, blk], z[:, :nbk, :],
                                axis=mybir.AxisListType.X, op=mybir.AluOpType.add,
                            )
                        if b2 != 0.0:
                            nc.vector.tensor_scalar_add(lg[:, : clen // 128],
                                                        lg[:, : clen // 128], b2)
                        lgb = p3.tile([128, c.CHUNK // 128], BF16, tag="lgb")
                        nc.vector.tensor_copy(lgb[:, : clen // 128],
                                              lg[:, : clen // 128])
                        nc.sync.dma_start(
                            logits_h[:, coff // 128 : (coff + clen) // 128],
                            lgb[:, : clen // 128],
                        )

    nc.compile()
    return nc


_BUILD_CACHE = {}


def _kernel_impl(inputs, cfg):
    in_maps, struct, orig_all = host_prep(inputs, cfg)
    key = (cfg.N, cfg.E, struct["TOT"], struct["NEF"], str(struct["pieces"]),
           struct["b2"])
    if key not in _BUILD_CACHE:
        _BUILD_CACHE.clear()
        _BUILD_CACHE[key] = build(cfg, struct)
    nc = _BUILD_CACHE[key]
    res = run_bass_kernel_spmd(nc, in_maps, list(range(cfg.NC)))
    out = np.empty(cfg.E, np.float32)
    flat = np.concatenate([res.results[k]["logits"].astype(np.float32).T.reshape(-1)
                           for k in range(cfg.NC)])
    valid = orig_all >= 0
    out[orig_all[valid]] = flat[valid]
    return out


def kernel(**inputs):
    cfg = CFG(N=100000, E=1_600_000, T=5, DIN=32, DH=32, EF=16)
    return _kernel_impl(inputs, cfg)


# revision 42
# speedup vs baseline: 1.5507x; 1.2630x over previous
"""Trainium2 Bass kernel for nn_EvolvingGNN (LSTM-evolved GCN + edge MLP).

Strategy (8 NeuronCores, full inputs in / full output out):
  - Nodes sharded 12500/core. Edges partitioned by destination core.
  - LSTM distributed: each core computes 512 of the 4096 gate rows
    (reads only its slice of W_ih/W_hh), AllGather of h each step.
  - xwd[n] = dinv[n] * (x[n] @ W) computed on the node shard, AllGathered
    into a full 256B-row table for gathers.
  - Message phase: dma_gather xwd[src] -> dma_scatter_add into agg[dst]
    (CCE add). Scatter calls must have unique indices (duplicate rows in
    one call race on read-modify-write), so edges are organised into
    "rounds" (r-th in-edge of each node) with round-robin over 4
    accumulator tables to hide the inter-round ordering latency.
  - emb = relu(dinv * (agg + xwd_self)); uv = [emb@W1a.T | emb@W1b.T]
    (one 256B row per node), AllGathered.
  - Edge MLP: gather uv[src] (u half) + uv[dst] (v half), w = ea@W1c.T+b1
    via PE matmuls on host-transposed edge_attr, logits = relu(z) . W2 + b2
    via DVE mul+reduce.
  - Gather indices are int16, so the node-table rows are bucketed in
    32768-row groups; the per-core edge order is (bucket, round, dst).
    Pads: gathers use row 0, scatters use a trash row.

Wall-clock optimisations (the axon tunnel moves ~60MB/s, so host->device
bytes dominate the end-to-end time):
  - Accumulator tables and uv_own are Internal DRAM zeroed on device
    (previously ExternalOutputs: ~13MB/core of donated zeros uploaded and
    ~16MB/core of unused outputs downloaded per call).
  - Big payloads (edge features, x, LSTM weights) travel as bfloat16;
    matmuls run bf16 x bf16 -> f32 PSUM.
  - Gather/scatter index planes are sent as the 16-partition master copy
    and replicated to the 128-partition layout on device (8x fewer bytes).
  - host_prep is vectorised: one combined-key argsort pipeline over all
    edges instead of per-core lexsorts.
"""

import os
import pickle
import subprocess
import sys

# Persistent XLA compilation cache: run_bass_kernel_spmd re-jits a fresh
# closure every call, so without this the neuronx compile hook re-runs
# (~0.7s/call) even though the lowered module is byte-identical.
os.environ.setdefault("JAX_COMPILATION_CACHE_DIR", "/tmp/jax_comp_cache")
os.environ.setdefault("JAX_PERSISTENT_CACHE_MIN_COMPILE_TIME_SECS", "0")
os.environ.setdefault("JAX_PERSISTENT_CACHE_MIN_ENTRY_SIZE_BYTES", "0")

import numpy as np
import ml_dtypes

import concourse.bacc as bacc

try:
    import jax as _jax
    _jax.config.update("jax_compilation_cache_dir", "/tmp/jax_comp_cache")
    _jax.config.update("jax_persistent_cache_min_compile_time_secs", 0)
    _jax.config.update("jax_persistent_cache_min_entry_size_bytes", 0)
except Exception:
    pass
import concourse.mybir as mybir
import concourse.tile as tile
from concourse.bass_utils import run_bass_kernel_spmd
from concourse.masks import make_identity

F32 = mybir.dt.float32
BF16 = mybir.dt.bfloat16
I16 = mybir.dt.int16
NPBF16 = ml_dtypes.bfloat16


class CFG:
    def __init__(self, N, E, T, DIN, DH, EF, NC=8, CHUNK=8192, CCH=4):
        self.N, self.E, self.T = N, E, T
        self.DIN, self.DH, self.EF = DIN, DH, EF
        self.FLAT = DIN * DH
        self.NC = NC
        assert N % NC == 0
        self.SH = N // NC                       # nodes per core
        self.TILES = -(-self.SH // 128)         # node tiles per core
        self.SHP = self.TILES * 128             # padded shard rows
        self.NTAB = NC * self.SHP               # full table rows
        self.NBUCK = -(-self.NTAB // 32768)
        self.CHUNK = CHUNK                      # gather chunk (edges)
        self.CCH = CCH                          # scatter chain tables
        self.ROW = 64                           # table row f32 (256B)
        # LSTM slicing: core k owns gate rows {g*FLAT + k*GSL + j}
        assert (4 * self.FLAT) % NC == 0
        self.GSL = self.FLAT // NC              # per-gate slice (128)
        self.KCH = self.FLAT // 128             # contraction chunks (8)


def _roundup(x, m):
    return -(-x // m) * m


# ---------------------------------------------------------------------------
# Parallel host prep: 8 numpy-only worker subprocesses over shared memory.
# Phase A: per-core edge sort into (bucket, round, dst) order + round counts.
# Phase B: slot assignment + assembly of idx planes / edge features / weights.
# ---------------------------------------------------------------------------

_WORKER_SRC = r"""
import sys, pickle
import numpy as np
import ml_dtypes
from multiprocessing import shared_memory

BF16 = ml_dtypes.bfloat16
_inp = sys.stdin.buffer
_out = sys.stdout.buffer
_shm = {}
_state = {}


def att(name):
    s = _shm.get(name)
    if s is None:
        s = shared_memory.SharedMemory(name=name, track=False)
        _shm[name] = s
    return s


def view(name, shape, dtype):
    n = int(np.prod(shape)) * np.dtype(dtype).itemsize
    return np.ndarray(shape, dtype, buffer=att(name).buf[:n])


while True:
    try:
        cmd = pickle.load(_inp)
    except EOFError:
        break
    op = cmd["op"]
    if op == "A":
        k = cmd["k"]; E = cmd["E"]; SH = cmd["SH"]; SHP = cmd["SHP"]
        NBUCK = cmd["NBUCK"]
        ei = view(cmd["ei"], (2, E), np.int32)
        dst = ei[1]
        lo = k * SH
        eids = np.flatnonzero((dst >= lo) & (dst < lo + SH))
        n = len(eids)
        s = ei[0][eids].astype(np.int64)
        rowid = (s // SH) * SHP + (s % SH)
        sbuck = (rowid >> 15).astype(np.int32)
        s16 = (rowid & 32767).astype(np.int16)
        dloc = (dst[eids] - lo).astype(np.int32)
        o1 = np.argsort(sbuck * np.int32(SH) + dloc, kind="stable")
        b1, d1 = sbuck[o1], dloc[o1]
        k1 = b1 * np.int32(SH) + d1
        newrun = np.empty(n, bool); newrun[:1] = True
        np.not_equal(k1[1:], k1[:-1], out=newrun[1:])
        starts = np.flatnonzero(newrun)
        r1 = (np.arange(n) - np.repeat(starts, np.diff(np.r_[starts, n]))).astype(np.int32)
        MAXR = int(r1.max()) + 1 if n else 1
        o2 = np.argsort((b1 * np.int32(MAXR) + r1) * np.int32(SH) + d1,
                        kind="stable")
        _state["eids"] = eids[o1][o2]
        _state["b"] = b1[o2]
        _state["r"] = r1[o2]
        _state["d"] = d1[o2]
        _state["s16"] = s16[o1][o2]
        _state["MAXR"] = MAXR
        cnt = np.bincount(_state["b"] * np.int32(MAXR) + _state["r"],
                          minlength=NBUCK * MAXR).reshape(NBUCK, MAXR)
        pickle.dump(cnt, _out); _out.flush()
    elif op == "B":
        k = cmd["k"]; E = cmd["E"]; TOT = cmd["TOT"]; EF = cmd["EF"]
        NEF = cmd["NEF"]; SHP = cmd["SHP"]
        seg_off = cmd["seg_off"]                       # [NBUCK, MAXR_glob]
        b, r, d = _state["b"], _state["r"], _state["d"]
        eids, s16, MAXR = _state["eids"], _state["s16"], _state["MAXR"]
        n = len(eids)
        ckey = b * np.int32(MAXR) + r
        newseg = np.empty(n, bool); newseg[:1] = True
        np.not_equal(ckey[1:], ckey[:-1], out=newseg[1:])
        sstarts = np.flatnonzero(newseg)
        rank = np.arange(n) - np.repeat(sstarts, np.diff(np.r_[sstarts, n]))
        slot = seg_off[b, r] + rank
        planes = view(cmd["planes"], (8, 2, 16, TOT // 16), np.int16)
        eaT = view(cmd["eaT"], (8, NEF, TOT), BF16)
        orig = view(cmd["orig"], (8, TOT), np.int32)
        ea = view(cmd["ea"], (E, EF), np.float32)
        u16 = np.zeros(TOT, np.int16); u16[slot] = s16
        vs = np.full(TOT, SHP, np.int16); vs[slot] = d.astype(np.int16)
        planes[k, 0] = u16.reshape(TOT // 16, 16).T
        planes[k, 1] = vs.reshape(TOT // 16, 16).T
        og = np.full(TOT, -1, np.int32); og[slot] = eids.astype(np.int32)
        orig[k] = og
        rows = np.zeros((TOT, NEF), BF16)
        rows[slot, : EF] = ea[eids].astype(BF16)
        if NEF > EF:
            rows[slot, EF] = 1.0
        eaT[k] = rows.T
        pickle.dump(k, _out); _out.flush()
"""


class _PrepPool:
    def __init__(self, n=8):
        self.n = n
        self.procs = [
            subprocess.Popen([sys.executable, "-u", "-c", _WORKER_SRC],
                             stdin=subprocess.PIPE, stdout=subprocess.PIPE)
            for _ in range(n)
        ]
        self.shms = {}

    def arr(self, key, shape, dtype):
        from multiprocessing import shared_memory
        nbytes = int(np.prod(shape)) * np.dtype(dtype).itemsize
        cur = self.shms.get(key)
        if cur is None or cur.size < nbytes:
            if cur is not None:
                try:
                    cur.close(); cur.unlink()
                except Exception:
                    pass
            cur = shared_memory.SharedMemory(create=True, size=nbytes)
            self.shms[key] = cur
        return np.ndarray(shape, dtype, buffer=cur.buf[:nbytes]), cur.name

    def send(self, i, obj):
        pickle.dump(obj, self.procs[i].stdin)
        self.procs[i].stdin.flush()

    def recv(self, i):
        return pickle.load(self.procs[i].stdout)

    def kill(self):
        for p in self.procs:
            try:
                p.kill()
            except Exception:
                pass
        for s in self.shms.values():
            try:
                s.close(); s.unlink()
            except Exception:
                pass


_POOL = None


def _get_pool():
    global _POOL
    if _POOL is None:
        _POOL = _PrepPool()
    return _POOL


def _host_prep_parallel(inputs, cfg):
    c = cfg
    pool = _get_pool()
    ei = np.asarray(inputs["edge_index"])
    ei_shm, ei_name = pool.arr("ei", (2, c.E), np.int32)
    np.copyto(ei_shm, ei)
    ea_shm, ea_name = pool.arr("ea", (c.E, c.EF), np.float32)
    np.copyto(ea_shm, np.asarray(inputs["edge_attr"], np.float32))
    for k in range(c.NC):
        pool.send(k, {"op": "A", "k": k, "E": c.E, "SH": c.SH, "SHP": c.SHP,
                      "NBUCK": c.NBUCK, "ei": ei_name})

    # parent-side smalls while workers sort
    wmat = _host_lstm(inputs, cfg)
    x_last = np.asarray(inputs["x"][-1], np.float32)
    xlT16 = x_last.T.astype(NPBF16)                             # [DIN, N]
    dst = ei_shm[1]
    deg = np.bincount(dst, minlength=c.N).astype(np.float32) + 1.0
    dinv = (1.0 / np.sqrt(deg)).astype(np.float32)
    W1 = np.asarray(inputs["W1"], np.float32)
    w1ab = np.ascontiguousarray(
        np.concatenate([W1[:, : c.DH].T, W1[:, c.DH : 2 * c.DH].T], axis=1))
    b1v = np.asarray(inputs["b1"], np.float32)
    has_b1 = bool(np.any(b1v))
    NEF = c.EF + 1 if has_b1 else c.EF
    w1c_parts = [W1[:, 2 * c.DH :].T] + ([b1v[None, :]] if has_b1 else [])
    w1c = np.ascontiguousarray(np.concatenate(w1c_parts).astype(NPBF16))
    w2 = np.asarray(inputs["W2"], np.float32).reshape(-1)
    w2row = np.ascontiguousarray(np.tile(w2, 512 // c.DH)[None, :])

    cnts = [pool.recv(k) for k in range(c.NC)]
    MAXR = max(cn.shape[1] for cn in cnts)
    segmax = np.zeros((c.NBUCK, MAXR), np.int64)
    for cn in cnts:
        np.maximum(segmax[:, : cn.shape[1]], cn, out=segmax[:, : cn.shape[1]])
    segsz = np.where(segmax > 0, ((segmax + 127) // 128) * 128, 0).astype(np.int64)
    seg_off = np.concatenate([[0], np.cumsum(segsz.reshape(-1))])[:-1].reshape(
        c.NBUCK, MAXR)
    TOT = int(segsz.sum())

    blen = segsz.sum(axis=1)
    bstarts = np.concatenate([[0], np.cumsum(blen)])
    pieces = []
    piece_ctr = 0
    for bb in range(c.NBUCK):
        bstart, bl = int(bstarts[bb]), int(blen[bb])
        if bl == 0:
            continue
        cuts = list(range(bstart, bstart + bl, c.CHUNK)) + [bstart + bl]
        for ci in range(len(cuts) - 1):
            coff, cend = cuts[ci], cuts[ci + 1]
            plist = []
            for rv in range(MAXR):
                if segsz[bb, rv] == 0:
                    continue
                so = int(seg_off[bb, rv])
                se = so + int(segsz[bb, rv])
                lo, hi = max(so, coff), min(se, cend)
                while lo < hi:
                    sub = min(hi - lo, 4096)
                    plist.append((lo - coff, sub, piece_ctr % c.CCH))
                    piece_ctr += 1
                    lo += sub
            pieces.append((bb, coff, cend - coff, plist))

    planes_shm, planes_name = pool.arr("planes", (8, 2, 16, TOT // 16), np.int16)
    eaT_shm, eaT_name = pool.arr("eaTo", (8, NEF, TOT), NPBF16)
    orig_shm, orig_name = pool.arr("orig", (8, TOT), np.int32)
    for k in range(c.NC):
        pool.send(k, {"op": "B", "k": k, "E": c.E, "TOT": TOT, "EF": c.EF,
                      "NEF": NEF, "SHP": c.SHP,
                      "seg_off": seg_off, "planes": planes_name,
                      "eaT": eaT_name, "orig": orig_name, "ea": ea_name})

    in_maps = []
    for k in range(c.NC):
        n0 = k * c.SH
        xT = np.zeros((c.DIN, c.SHP), NPBF16)
        xT[:, : c.SH] = xlT16[:, n0 : n0 + c.SH]
        dflat = np.ones(c.SHP, np.float32)
        dflat[: c.SH] = dinv[n0 : n0 + c.SH]
        dvt = np.ascontiguousarray(dflat.reshape(c.TILES, 128).T)
        in_maps.append({
            "xT": xT, "dinv": dvt, "wmat": wmat,
            "w1ab": w1ab, "w1c": w1c, "w2row": w2row,
            "uidx": planes_shm[k, 0], "vsidx": planes_shm[k, 1],
            "eaT": eaT_shm[k],
        })
    for k in range(c.NC):
        pool.recv(k)

    struct = {
        "TOT": TOT,
        "NEF": NEF,
        "pieces": pieces,
        "b2": float(np.asarray(inputs["b2"], np.float32).reshape(-1)[0]),
    }
    return in_maps, struct, orig_shm.reshape(-1)


def _ncpu():
    try:
        return len(os.sched_getaffinity(0))
    except Exception:
        return os.cpu_count() or 1


def _host_lstm(inputs, cfg):
    """The weight-evolving LSTM depends only on (tiny) host-known inputs —
    42 MFLOP of serial matvecs. Run it on host in f32 (exact vs reference)
    instead of shipping 33MB of LSTM weights through the slow tunnel."""
    c = cfg
    W_ih = np.asarray(inputs["W_ih"], np.float32)
    W_hh = np.asarray(inputs["W_hh"], np.float32)
    b = (np.asarray(inputs["b_ih"], np.float32)
         + np.asarray(inputs["b_hh"], np.float32))
    inp = np.asarray(inputs["initial_weights"], np.float32).reshape(-1)
    h = np.zeros(c.FLAT, np.float32)
    cs = np.zeros(c.FLAT, np.float32)
    for _ in range(c.T):
        gates = W_ih @ inp + W_hh @ h + b
        i, f, g, o = np.split(gates, 4)
        i = 1.0 / (1.0 + np.exp(-i))
        f = 1.0 / (1.0 + np.exp(-f))
        g = np.tanh(g)
        o = 1.0 / (1.0 + np.exp(-o))
        cs = f * cs + i * g
        h = o * np.tanh(cs)
        inp = h
    return np.ascontiguousarray(h.reshape(c.DIN, c.DH).astype(NPBF16))


def host_prep(inputs, cfg):
    # The worker pool only pays off with real parallelism; on the 1-2 CPU
    # containers the serial vectorised path is strictly better.
    global _POOL
    if os.environ.get("KPREP_SERIAL") != "1" and _ncpu() >= 4:
        try:
            return _host_prep_parallel(inputs, cfg)
        except Exception:
            if _POOL is not None:
                _POOL.kill()
                _POOL = None
    return _host_prep_serial(inputs, cfg)


def _host_prep_serial(inputs, cfg):
    """Shard / reorder everything on the host. Returns (in_maps, struct, origs)."""
    c = cfg
    x_last = np.asarray(inputs["x"][-1], np.float32)            # [N, DIN]
    ei = np.asarray(inputs["edge_index"])                       # [2, E]
    ea = np.asarray(inputs["edge_attr"], np.float32)            # [E, EF]
    src = ei[0].astype(np.int32)
    dst = ei[1].astype(np.int32)

    deg = np.bincount(dst, minlength=c.N).astype(np.float32) + 1.0
    dinv = (1.0 / np.sqrt(deg)).astype(np.float32)

    rowid = (src // c.SH) * c.SHP + (src % c.SH)                # table row of src
    sbuck = rowid >> 15
    s16 = (rowid & 32767).astype(np.int16)
    ecore = dst // c.SH
    dloc = dst - ecore * c.SH

    # ---- global (core, bucket, round, dst) ordering ----
    key1 = (ecore * c.NBUCK + sbuck) * c.SH + dloc              # int32
    o1 = np.argsort(key1, kind="stable").astype(np.int32)
    k1 = key1[o1]
    newrun = np.empty(c.E, bool)
    newrun[0] = True
    np.not_equal(k1[1:], k1[:-1], out=newrun[1:])
    starts = np.flatnonzero(newrun).astype(np.int32)
    ar = np.arange(c.E, dtype=np.int32)
    r1 = ar - np.repeat(starts, np.diff(np.r_[starts, np.int32(c.E)]))
    MAXR = int(r1.max()) + 1
    key2 = ((ecore[o1] * c.NBUCK + sbuck[o1]) * np.int32(MAXR) + r1) * c.SH \
        + dloc[o1]
    o2 = np.argsort(key2, kind="stable").astype(np.int32)
    eid2 = o1[o2]
    k2s = key2[o2]
    ckey = k2s // c.SH                      # (ec*NBUCK + b)*MAXR + r
    d2 = k2s - ckey * c.SH                  # dloc

    # ---- universal segment sizes: max count over cores per (bucket, round) ----
    cnt = np.bincount(ckey, minlength=c.NC * c.NBUCK * MAXR).reshape(
        c.NC, c.NBUCK, MAXR)
    segmax = cnt.max(axis=0)                                    # [NBUCK, MAXR]
    segsz = np.where(segmax > 0, ((segmax + 127) // 128) * 128, 0).astype(np.int64)
    seg_off = np.concatenate([[0], np.cumsum(segsz.reshape(-1))])[:-1].reshape(
        c.NBUCK, MAXR).astype(np.int32)
    TOT = int(segsz.sum())
    assert TOT % 128 == 0

    # ---- per-edge slot ----
    newseg = np.empty(c.E, bool)
    newseg[0] = True
    np.not_equal(ckey[1:], ckey[:-1], out=newseg[1:])
    sstarts = np.flatnonzero(newseg).astype(np.int32)
    rank = ar - np.repeat(sstarts, np.diff(np.r_[sstarts, np.int32(c.E)]))
    br = ckey % np.int32(c.NBUCK * MAXR)    # b*MAXR + r
    ec2 = ckey // np.int32(c.NBUCK * MAXR)
    slot = seg_off.reshape(-1)[br] + rank                       # [0, TOT) per core
    gslot = ec2 * np.int32(TOT) + slot

    # ---- chunk / scatter-piece structure (identical for all cores) ----
    blen = segsz.sum(axis=1)                                    # per bucket
    bstarts = np.concatenate([[0], np.cumsum(blen)])
    pieces = []                                                 # (bb,coff,clen,[(po,pl,chain)])
    piece_ctr = 0
    for bb in range(c.NBUCK):
        bstart, bl = int(bstarts[bb]), int(blen[bb])
        if bl == 0:
            continue
        cuts = list(range(bstart, bstart + bl, c.CHUNK)) + [bstart + bl]
        for ci in range(len(cuts) - 1):
            coff, cend = cuts[ci], cuts[ci + 1]
            plist = []
            for rv in range(MAXR):
                if segsz[bb, rv] == 0:
                    continue
                so = int(seg_off[bb, rv])
                se = so + int(segsz[bb, rv])
                lo, hi = max(so, coff), min(se, cend)
                # dma_scatter_add breaks above 4096 idxs per call
                while lo < hi:
                    sub = min(hi - lo, 4096)
                    plist.append((lo - coff, sub, piece_ctr % c.CCH))
                    piece_ctr += 1
                    lo += sub
            pieces.append((bb, coff, cend - coff, plist))

    # ---- global slot-order tables ----
    TRASH = c.SHP                                               # scatter/v pad row
    NT = c.NC * TOT
    u16_all = np.zeros(NT, np.int16)
    u16_all[gslot] = s16[eid2]
    vs_all = np.full(NT, TRASH, np.int16)
    vs_all[gslot] = d2.astype(np.int16)
    orig_all = np.full(NT, -1, np.int32)
    orig_all[gslot] = eid2

    b1v = np.asarray(inputs["b1"], np.float32)
    has_b1 = bool(np.any(b1v))
    NEF = c.EF + 1 if has_b1 else c.EF
    ea16 = ea.astype(NPBF16)
    ea_rows = np.zeros((NT, NEF), NPBF16)
    ea_rows[gslot, : c.EF] = ea16[eid2]
    if has_b1:
        ea_rows[gslot, c.EF] = 1.0

    xlT16 = x_last.T.astype(NPBF16)                             # [DIN, N]

    W1 = np.asarray(inputs["W1"], np.float32)                   # [DH, 2DH+EF]
    w1ab = np.ascontiguousarray(
        np.concatenate([W1[:, : c.DH].T, W1[:, c.DH : 2 * c.DH].T], axis=1))
    w1c_parts = [W1[:, 2 * c.DH :].T] + ([b1v[None, :]] if has_b1 else [])
    w1c = np.ascontiguousarray(np.concatenate(w1c_parts).astype(NPBF16))
    w2 = np.asarray(inputs["W2"], np.float32).reshape(-1)       # [DH]
    w2row = np.ascontiguousarray(np.tile(w2, 512 // c.DH)[None, :])  # [1, 512]
    wmat = _host_lstm(inputs, cfg)                              # [DIN, DH] bf16

    in_maps = []
    for k in range(c.NC):
        sl = slice(k * TOT, (k + 1) * TOT)
        n0 = k * c.SH

        xT = np.zeros((c.DIN, c.SHP), NPBF16)
        xT[:, : c.SH] = xlT16[:, n0 : n0 + c.SH]
        dflat = np.ones(c.SHP, np.float32)
        dflat[: c.SH] = dinv[n0 : n0 + c.SH]
        dvt = np.ascontiguousarray(dflat.reshape(c.TILES, 128).T)

        in_maps.append({
            "xT": xT,
            "dinv": dvt,
            "wmat": wmat,
            "w1ab": w1ab,
            "w1c": w1c,
            "w2row": w2row,
            "uidx": np.ascontiguousarray(u16_all[sl].reshape(TOT // 16, 16).T),
            "vsidx": np.ascontiguousarray(vs_all[sl].reshape(TOT // 16, 16).T),
            "eaT": np.ascontiguousarray(ea_rows[sl].T),         # [NEF, TOT] bf16
        })

    struct = {
        "TOT": TOT,
        "NEF": NEF,
        "pieces": pieces,
        "b2": float(np.asarray(inputs["b2"], np.float32).reshape(-1)[0]),
    }
    return in_maps, struct, orig_all


def build(cfg, struct):
    c = cfg
    TOT = struct["TOT"]
    NEF = struct["NEF"]
    nc = bacc.Bacc("TRN2", target_bir_lowering=False, debug=False,
                   num_devices=c.NC)

    # ---------- I/O ----------
    xT_h = nc.dram_tensor("xT", [c.DIN, c.SHP], BF16, kind="ExternalInput")
    dinv_h = nc.dram_tensor("dinv", [128, c.TILES], F32, kind="ExternalInput")
    wmat_h = nc.dram_tensor("wmat", [c.DIN, c.DH], BF16, kind="ExternalInput")
    w1ab_h = nc.dram_tensor("w1ab", [c.DH, 2 * c.DH], F32, kind="ExternalInput")
    w1c_h = nc.dram_tensor("w1c", [NEF, c.DH], BF16, kind="ExternalInput")
    w2row_h = nc.dram_tensor("w2row", [1, 512], F32, kind="ExternalInput")
    uidx_h = nc.dram_tensor("uidx", [16, TOT // 16], I16, kind="ExternalInput")
    vsidx_h = nc.dram_tensor("vsidx", [16, TOT // 16], I16, kind="ExternalInput")
    eaT_h = nc.dram_tensor("eaT", [NEF, TOT], BF16, kind="ExternalInput")

    logits_h = nc.dram_tensor("logits", [128, TOT // 128], BF16, kind="ExternalOutput")
    # internal accumulator tables, zeroed on device before the scatter phase
    aggs = [nc.dram_tensor(f"agg{i}", [c.SHP + 128, c.ROW], F32)
            for i in range(c.CCH)]
    uv_own = nc.dram_tensor("uv_own", [c.SHP + 128, c.ROW], F32)

    # internal DRAM
    xwd_own = nc.dram_tensor("xwd_own", [c.SHP, c.ROW], F32)
    xwd_full = nc.dram_tensor("xwd_full", [c.NTAB, c.ROW], F32, addr_space="Shared")
    uv_shard = nc.dram_tensor("uv_shard", [c.SHP, c.ROW], F32)
    uv_full = nc.dram_tensor("uv_full", [c.NTAB, c.ROW], F32, addr_space="Shared")

    groups = [list(range(c.NC))]

    with tile.TileContext(nc) as tc:
        with (
            tc.tile_pool(name="persist", bufs=1) as pp,
            tc.tile_pool(name="psum_ls", bufs=2, space="PSUM") as ps_ls,
        ):
            # ---------- persistent small tiles ----------
            ident = pp.tile([128, 128], F32)
            make_identity(nc, ident[:])
            w1ab_sb = pp.tile([c.DH, 2 * c.DH], F32)
            nc.sync.dma_start(w1ab_sb[:], w1ab_h[:])
            w1c_sb = pp.tile([NEF, c.DH], BF16)
            nc.sync.dma_start(w1c_sb[:], w1c_h[:])
            dinv_sb = pp.tile([128, c.TILES], F32)
            nc.sync.dma_start(dinv_sb[:], dinv_h[:])
            xwd_sb = pp.tile([128, c.TILES, c.DH], F32)  # persists to post-agg
            W_sb = pp.tile([c.DIN, c.DH], BF16)          # evolved GCN weight
            nc.sync.dma_start(W_sb[:], wmat_h[:])

            # w2 broadcast [1,512] -> [128,512] via K=1 matmul with ones
            w2r_sb = pp.tile([1, 512], F32)
            nc.sync.dma_start(w2r_sb[:], w2row_h[:])
            ones1 = pp.tile([1, 128], F32)
            nc.vector.memset(ones1[:], 1.0)
            w2_sb = pp.tile([128, 512], F32)
            pw2 = ps_ls.tile([128, 512], F32, tag="w2bc")
            nc.tensor.matmul(pw2[:], ones1[:], w2r_sb[:], start=True, stop=True)
            nc.vector.tensor_copy(w2_sb[:], pw2[:])

            # ---------- zero the accumulator tables (device-side) ----------
            zt = pp.tile([128, 16, c.ROW], F32)
            nc.vector.memset(zt[:], 0.0)
            ntile = (c.SHP + 128) // 128
            for t in aggs:
                av = t[:, :].rearrange("(x p) c -> p x c", p=128)
                for x0 in range(0, ntile, 16):
                    xl = min(16, ntile - x0)
                    nc.sync.dma_start(av[:, x0 : x0 + xl, :], zt[:, :xl, :])
            nc.sync.dma_start(uv_own[c.SHP : c.SHP + 128, :], zt[:, 0, :])

            # ---------- phase B: xwd = dinv * (x @ W) ----------
            with (
                tc.tile_pool(name="xw", bufs=3) as xp,
                tc.tile_pool(name="psum_xw", bufs=4, space="PSUM") as ps_xw,
            ):
                xT_sb = xp.tile([c.DIN, c.SHP], BF16, tag="xT")
                nc.sync.dma_start(xT_sb[:], xT_h[:])
                for t in range(c.TILES):
                    pxw = ps_xw.tile([128, c.DH], F32, tag="pxw")
                    nc.tensor.matmul(pxw[:], xT_sb[:, t * 128 : (t + 1) * 128],
                                     W_sb[:], start=True, stop=True)
                    nc.vector.tensor_scalar(
                        xwd_sb[:, t, :], pxw[:], dinv_sb[:, t : t + 1], None,
                        op0=mybir.AluOpType.mult,
                    )
                    nc.sync.dma_start(
                        xwd_own[t * 128 : (t + 1) * 128, 0 : c.DH],
                        xwd_sb[:, t, :],
                    )

            tc.strict_bb_all_engine_barrier()
            nc.gpsimd.collective_compute(
                "AllGather", mybir.AluOpType.bypass,
                replica_groups=groups,
                ins=[xwd_own[:, :].opt()],
                outs=[xwd_full[:, :].opt()],
            )
            tc.strict_bb_all_engine_barrier()

            # ---------- idx planes: replicate 16-row master to 128 partitions ----
            with tc.tile_pool(name="planes", bufs=1) as plp:
                up = plp.tile([128, TOT // 16], I16)
                vp = plp.tile([128, TOT // 16], I16)
                for g in range(8):
                    nc.sync.dma_start(up[16 * g : 16 * (g + 1), :], uidx_h[:, :])
                    nc.sync.dma_start(vp[16 * g : 16 * (g + 1), :], vsidx_h[:, :])

                # ---------- phase 1: gather msgs + scatter-add ----------
                with tc.tile_pool(name="p1", bufs=3) as p1:
                    for bb, coff, clen, plist in struct["pieces"]:
                        msg = p1.tile([128, c.CHUNK // 128, c.ROW], F32, tag="msg")
                        nc.gpsimd.dma_gather(
                            msg[:, : clen // 128, :],
                            xwd_full[bb * 32768 :, :],
                            up[:, coff // 16 : (coff + clen) // 16],
                            clen, clen, c.ROW, single_packet=False,
                        )
                        for po, pl, chain in plist:
                            nc.gpsimd.dma_scatter_add(
                                aggs[chain][:, :],
                                msg[:, po // 128 : (po + pl) // 128, :],
                                vp[:, (coff + po) // 16 : (coff + po + pl) // 16],
                                pl, pl, c.ROW, single_packet=False,
                            )

                tc.strict_bb_all_engine_barrier()

                # ---------- phase 2: emb, uv tables ----------
                with (
                    tc.tile_pool(name="p2", bufs=3) as p2,
                    tc.tile_pool(name="psum_t", bufs=2, space="PSUM") as ps_t,
                    tc.tile_pool(name="psum_uv", bufs=2, space="PSUM") as ps_uv,
                ):
                    for t in range(c.TILES):
                        r0, r1 = t * 128, (t + 1) * 128
                        ag = [p2.tile([128, c.ROW], F32, tag=f"ag{i}", name=f"ag{i}")
                              for i in range(c.CCH)]
                        for i in range(c.CCH):
                            nc.sync.dma_start(ag[i][:], aggs[i][r0:r1, :])
                        s0 = p2.tile([128, c.DH], F32, tag="s0")
                        s1 = p2.tile([128, c.DH], F32, tag="s1")
                        nc.vector.tensor_tensor(s0[:], ag[0][:, : c.DH], ag[1][:, : c.DH],
                                                op=mybir.AluOpType.add)
                        nc.vector.tensor_tensor(s1[:], ag[2][:, : c.DH], ag[3][:, : c.DH],
                                                op=mybir.AluOpType.add)
                        nc.vector.tensor_tensor(s0[:], s0[:], s1[:],
                                                op=mybir.AluOpType.add)
                        nc.vector.tensor_tensor(s0[:], s0[:], xwd_sb[:, t, :],
                                                op=mybir.AluOpType.add)
                        emb = p2.tile([128, c.DH], F32, tag="emb")
                        nc.scalar.activation(emb[:], s0[:],
                                             mybir.ActivationFunctionType.Relu,
                                             scale=dinv_sb[:, t : t + 1])
                        pt = ps_t.tile([c.DH, 128], F32, tag="pt")
                        nc.tensor.transpose(pt[:], emb[:], ident[:])
                        embT = p2.tile([c.DH, 128], F32, tag="embT")
                        nc.vector.tensor_copy(embT[:], pt[:])
                        puv = ps_uv.tile([128, 2 * c.DH], F32, tag="puv")
                        nc.tensor.matmul(puv[:], embT[:], w1ab_sb[:],
                                         start=True, stop=True)
                        uvt = p2.tile([128, c.ROW], F32, tag="uvt")
                        nc.vector.tensor_copy(uvt[:, : 2 * c.DH], puv[:])
                        nc.sync.dma_start(uv_own[r0:r1, :], uvt[:])
                        nc.sync.dma_start(uv_shard[r0:r1, :], uvt[:])

                tc.strict_bb_all_engine_barrier()
                nc.gpsimd.collective_compute(
                    "AllGather", mybir.AluOpType.bypass,
                    replica_groups=groups,
                    ins=[uv_shard[:, :].opt()],
                    outs=[uv_full[:, :].opt()],
                )
                tc.strict_bb_all_engine_barrier()

                # ---------- phase 3: edge MLP ----------
                b2 = struct["b2"]
                with (
                    tc.tile_pool(name="p3", bufs=2) as p3,
                    tc.tile_pool(name="psum_w", bufs=4, space="PSUM") as ps_w,
                ):
                    for bb, coff, clen, _pl in struct["pieces"]:
                        ug = p3.tile([128, c.CHUNK // 128, c.ROW], F32, tag="ug")
                        vg = p3.tile([128, c.CHUNK // 128, c.ROW], F32, tag="vg")
                        nc.gpsimd.dma_gather(
                            ug[:, : clen // 128, :], uv_full[bb * 32768 :, :],
                            up[:, coff // 16 : (coff + clen) // 16],
                            clen, clen, c.ROW, single_packet=False,
                        )
                        nc.gpsimd.dma_gather(
                            vg[:, : clen // 128, :], uv_own[:, :],
                            vp[:, coff // 16 : (coff + clen) // 16],
                            clen, clen, c.ROW, single_packet=False,
                        )
                        eat = p3.tile([NEF, c.CHUNK], BF16, tag="eat")
                        nc.sync.dma_start(eat[:, :clen],
                                          eaT_h[:, coff : coff + clen])
                        lg = p3.tile([128, c.CHUNK // 128], F32, tag="lg")
                        ngrp = -(-clen // 2048)
                        for g in range(ngrp):
                            e0 = g * 2048
                            gl = min(2048, clen - e0)               # multiple of 128
                            nbk = gl // 128
                            pw = ps_w.tile([128, 512], F32, tag="pw")
                            for e in range(nbk):
                                nc.tensor.matmul(
                                    pw[:, e * c.DH : (e + 1) * c.DH],
                                    eat[:, e0 + e * 128 : e0 + (e + 1) * 128],
                                    w1c_sb[:], start=True, stop=True,
                                )
                            z = p3.tile([128, 16, c.DH], F32, tag="z")
                            blk = slice(e0 // 128, e0 // 128 + nbk)
                            nc.vector.tensor_tensor(
                                z[:, :nbk, :], ug[:, blk, : c.DH],
                                vg[:, blk, c.DH : 2 * c.DH], op=mybir.AluOpType.add,
                            )
                            nc.vector.tensor_tensor(
                                z[:].rearrange("p a b -> p (a b)")[:, : nbk * c.DH],
                                z[:].rearrange("p a b -> p (a b)")[:, : nbk * c.DH],
                                pw[:, : nbk * c.DH],
                                op=mybir.AluOpType.add,
                            )
                            nc.scalar.activation(
                                z[:, :nbk, :], z[:, :nbk, :],
                                mybir.ActivationFunctionType.Relu,
                            )
                            nc.vector.tensor_tensor(
                                z[:, :nbk, :], z[:, :nbk, :],
                                w2_sb[:].rearrange("p (a b) -> p a b", b=c.DH)[:, :nbk, :],
                                op=mybir.AluOpType.mult,
                            )
                            nc.vector.tensor_reduce(
                                lg[:, blk], z[:, :nbk, :],
                                axis=mybir.AxisListType.X, op=mybir.AluOpType.add,
                            )
                        if b2 != 0.0:
                            nc.vector.tensor_scalar_add(lg[:, : clen // 128],
                                                        lg[:, : clen // 128], b2)
                        lgb = p3.tile([128, c.CHUNK // 128], BF16, tag="lgb")
                        nc.vector.tensor_copy(lgb[:, : clen // 128],
                                              lg[:, : clen // 128])
                        nc.sync.dma_start(
                            logits_h[:, coff // 128 : (coff + clen) // 128],
                            lgb[:, : clen // 128],
                        )

    nc.compile()
    return nc


_BUILD_CACHE = {}


def _kernel_impl(inputs, cfg):
    in_maps, struct, orig_all = host_prep(inputs, cfg)
    key = (cfg.N, cfg.E, struct["TOT"], struct["NEF"], str(struct["pieces"]),
           struct["b2"])
    if key not in _BUILD_CACHE:
        _BUILD_CACHE.clear()
        _BUILD_CACHE[key] = build(cfg, struct)
    nc = _BUILD_CACHE[key]
    res = run_bass_kernel_spmd(nc, in_maps, list(range(cfg.NC)))
    out = np.empty(cfg.E, np.float32)
    flat = np.concatenate([res.results[k]["logits"].astype(np.float32).T.reshape(-1)
                           for k in range(cfg.NC)])
    valid = orig_all >= 0
    out[orig_all[valid]] = flat[valid]
    return out


def kernel(**inputs):
    cfg = CFG(N=100000, E=1_600_000, T=5, DIN=32, DH=32, EF=16)
    return _kernel_impl(inputs, cfg)
